# revision 20
# baseline (speedup 1.0000x reference)
"""Trainium2 Bass kernel for nn_AttentionTD (3-block deformable attention TD).

Self-contained: hardcodes all shapes. Data-parallel over batch B=8 across the
8 NeuronCores; each core runs the full 3-block DAT stack for one batch element.

v3: precise offset path (f32r qproj + hi/lo bf16 depthwise conv), fp8
DoubleRow bias matmuls folding the fy-interp into the PE, head-pair packed AV
psum, 1024-wide exp chunks, scalar-engine reciprocal, phase-interleaved
emission across the three independent blocks.
"""

import sys

sys.path.insert(0, "/opt/trn_rl_repo")

import numpy as np

# ---------------- problem constants ----------------
B, C, H, W = 8, 128, 64, 64
NCH = 64          # channels per DAT block
NH, HC = 4, 16    # heads, head channels
KS = 4
HWS = H * W       # 4096
HK = WK = 16
NS = HK * WK      # 256 sample points
EPS = 1e-5
NBLK = 3
# rpe slice table geometry: [blk][h][x0 (64)][row (128)][col (65)]
TROW, TCOL = 128, 65
TSLICE = TROW * TCOL          # 8320
THEAD = 64 * TSLICE           # per (blk,h)
TBLK = NH * THEAD
NTAB = NBLK * TBLK

_CACHE = {}


def _build_graph():
    from concourse import bacc, mybir, tile
    import concourse.bass as bass
    from concourse.bass import IndirectOffsetOnAxis

    f32 = mybir.dt.float32
    f32r = mybir.dt.float32r
    bf16 = mybir.dt.bfloat16
    fp8 = mybir.dt.float8e4
    i32 = mybir.dt.int32
    Alu = mybir.AluOpType
    Act = mybir.ActivationFunctionType
    DR = mybir.MatmulPerfMode.DoubleRow

    nc = bacc.Bacc("TRN2", target_bir_lowering=False, debug=False, num_devices=8)

    # ---- dram io ----
    xi1_d = nc.dram_tensor("xi1", [C, HWS], f32, kind="ExternalInput").ap()
    xi2_d = nc.dram_tensor("xi2", [C, HWS], f32, kind="ExternalInput").ap()
    xq1_d = nc.dram_tensor("xq1", [64, HWS], f32r, kind="ExternalInput").ap()
    xq2_d = nc.dram_tensor("xq2", [64, HWS], f32r, kind="ExternalInput").ap()
    kvT0_d = nc.dram_tensor("kvT0", [HWS, NCH], f32, kind="ExternalInput").ap()
    kvT1_d = nc.dram_tensor("kvT1", [HWS, NCH], f32, kind="ExternalInput").ap()
    wpf_d = nc.dram_tensor("wpf", [64, 3 * 128], f32r, kind="ExternalInput").ap()
    wpb_d = nc.dram_tensor("wpb", [65, 3 * 192], bf16, kind="ExternalInput").ap()
    cp_d = nc.dram_tensor("cp", [128, 590], f32, kind="ExternalInput").ap()
    cpb_d = nc.dram_tensor("cpb", [128, 320], bf16, kind="ExternalInput").ap()
    cdg_d = nc.dram_tensor("cdg", [128, 3 * 16 * 128], bf16, kind="ExternalInput").ap()
    cdgl_d = nc.dram_tensor("cdgl", [128, 3 * 16 * 128], bf16, kind="ExternalInput").ap()
    tab_d = nc.dram_tensor("rpetab", [2 * NTAB, 1], fp8, kind="ExternalInput").ap()
    o1_d = nc.dram_tensor("o1", [C, HWS], f32, kind="ExternalOutput").ap()
    o2_d = nc.dram_tensor("o2", [C, HWS], f32, kind="ExternalOutput").ap()

    with tile.TileContext(nc) as tc:
        import contextlib

        ctx = contextlib.ExitStack()
        with ctx:
            cpool = ctx.enter_context(tc.tile_pool(name="const", bufs=1))
            xpool = ctx.enter_context(tc.tile_pool(name="xdata", bufs=1))
            qpool = ctx.enter_context(tc.tile_pool(name="qtiles", bufs=3))
            lpool = ctx.enter_context(tc.tile_pool(name="qlo", bufs=2))
            spool = ctx.enter_context(tc.tile_pool(name="stage", bufs=2))
            ppool = ctx.enter_context(tc.tile_pool(name="probs", bufs=1))
            apool = ctx.enter_context(tc.tile_pool(name="avs", bufs=2))
            gpool = ctx.enter_context(tc.tile_pool(name="wins", bufs=2))
            sb3 = ctx.enter_context(tc.tile_pool(name="blk", bufs=1))
            sbs2 = ctx.enter_context(tc.tile_pool(name="blkstate", bufs=2))
            sba = ctx.enter_context(tc.tile_pool(name="accs", bufs=3))
            sbt = ctx.enter_context(tc.tile_pool(name="tails", bufs=2))
            qkps = ctx.enter_context(tc.tile_pool(name="qk", bufs=2, space="PSUM"))
            avps = ctx.enter_context(tc.tile_pool(name="av", bufs=2, space="PSUM"))
            tps = ctx.enter_context(tc.tile_pool(name="tailp", bufs=2, space="PSUM"))

            # ---- persistent loads ----
            cp = cpool.tile([128, 590], f32, tag="cp")
            nc.sync.dma_start(out=cp[:, :], in_=cp_d)
            wpf = cpool.tile([64, 3 * 128], f32r, tag="wpf")
            nc.sync.dma_start(out=wpf[:, :], in_=wpf_d)
            wpb = cpool.tile([65, 3 * 192], bf16, tag="wpb")
            nc.sync.dma_start(out=wpb[:, :], in_=wpb_d)
            cpb = cpool.tile([128, 320], bf16, tag="cpb")
            nc.sync.dma_start(out=cpb[:, :], in_=cpb_d)
            cdg = cpool.tile([128, 3 * 16 * 128], bf16, tag="cdg")
            nc.sync.dma_start(out=cdg[:, :], in_=cdg_d)
            cdgl = cpool.tile([128, 3 * 16 * 128], bf16, tag="cdgl")
            nc.sync.dma_start(out=cdgl[:, :], in_=cdgl_d)
            xi1 = xpool.tile([C, HWS], f32, tag="xi1")
            nc.sync.dma_start(out=xi1[:, :], in_=xi1_d)
            xi2 = xpool.tile([C, HWS], f32, tag="xi2")
            nc.sync.dma_start(out=xi2[:, :], in_=xi2_d)

            zb = cpool.tile([128, 1], f32, tag="zb")
            nc.vector.memset(zb[:, :], 0.0)
            epst = cpool.tile([1, 1], f32, tag="epst")
            nc.vector.memset(epst[:, :], EPS)

            eye = cp[:, 0:128]
            ref_yx = cp[0:2, 128:384]          # row0 = y, row1 = x
            ones1_128 = cp[0:1, 384:512]       # [1,128] ones (bcast lhsT)
            ones128_div = cp[0:128, 520:521]   # 1/64 on data rows, 0 on gaps

            def act_raw(out, in_, func):
                eng = nc.scalar
                ins = [eng.lower_ap(in_)]
                for v in (0.0, 1.0, 0.0):
                    ins.append(mybir.ImmediateValue(dtype=mybir.dt.float32, value=v))
                return eng.add_instruction(
                    mybir.InstActivation(
                        name=nc.get_next_instruction_name(), func=func,
                        ins=ins, outs=[eng.lower_ap(out)],
                    )
                )

            def wf(blk, lo, hi):
                return wpf[:, blk * 128 + lo : blk * 128 + hi]

            def wb(blk, lo, hi, rows=64):
                return wpb[0:rows, blk * 192 + lo : blk * 192 + hi]

            BLKS = [(xq1_d, kvT0_d, xi1), (xq2_d, kvT0_d, xi2), (xq2_d, kvT1_d, xi2)]

            # ============ Phase A2: q projections (f32r via staging) ========
            q_bs = [None, None, None]
            q_ls = [None, None, None]

            def emit_qproj(blk):
                XQ_d = BLKS[blk][0]
                pq_wT_sp = wf(blk, 0, 128)
                pq_b_sp = cp[:, 521 + blk : 522 + blk]
                q_b = qpool.tile([128, HWS], bf16, tag="qb")
                q_l = lpool.tile([128, HWS], bf16, tag="ql")
                q_bs[blk] = q_b
                q_ls[blk] = q_l
                for mc in range(8):
                    stg = spool.tile([64, 512], f32r, tag="stg")
                    nc.sync.dma_start(out=stg[:, :], in_=XQ_d[:, mc * 512 : (mc + 1) * 512])
                    if mc % 2 == 0:
                        qp = qkps.tile([128, 512], f32, tag="qkp")
                    else:
                        qp = tps.tile([128, 512], f32, tag="tl")
                    nc.tensor.matmul(
                        out=qp[:, :], lhsT=pq_wT_sp, rhs=stg[:, :],
                        start=True, stop=True,
                    )
                    nc.scalar.activation(
                        out=q_b[:, mc * 512 : (mc + 1) * 512], in_=qp[:, :],
                        func=Act.Identity, bias=pq_b_sp,
                    )
                    # q_lo = (psum + bias) - q_b  (bf16 residual)
                    nc.vector.scalar_tensor_tensor(
                        out=q_l[:, mc * 512 : (mc + 1) * 512], in0=qp[:, :],
                        scalar=pq_b_sp, in1=q_b[:, mc * 512 : (mc + 1) * 512],
                        op0=Alu.add, op1=Alu.subtract,
                    )

            # ============ Phase A3: depthwise conv (PE diag, hi/lo) =========
            acc_ss = [None, None, None]

            def emit_conv(blk):
                q_b, q_l = q_bs[blk], q_ls[blk]
                q5 = q_b[:, :].rearrange("p (hh a ww b) -> p hh a ww b", hh=16, a=4, ww=16, b=4)
                q5l = q_l[:, :].rearrange("p (hh a ww b) -> p hh a ww b", hh=16, a=4, ww=16, b=4)
                acc_ps = tps.tile([128, NS], f32, tag="tl")
                first = True
                for grp, (lhs, rhsview) in enumerate(((cdg, q5), (cdgl, q5), (cdg, q5l))):
                    for t in range(16):
                        dy, dx = t // 4, t % 4
                        nc.tensor.matmul(
                            out=acc_ps[:, :],
                            lhsT=lhs[:, (blk * 16 + t) * 128 : (blk * 16 + t + 1) * 128],
                            rhs=rhsview[:, :, dy, :, dx],
                            start=first, stop=(grp == 2 and t == 15),
                        )
                        first = False
                dw_b = cp[:, 527 + blk * 21 + 16 : 527 + blk * 21 + 17]
                acc_s = sba.tile([128, NS], f32, tag="dwacc")
                acc_ss[blk] = acc_s
                nc.vector.tensor_scalar(
                    out=acc_s[:, :], in0=acc_ps[:, :], scalar1=dw_b, scalar2=None, op0=Alu.add
                )

            # ============ per-block prologue (LN, GELU, offsets, idx) =======
            def emit_prologue(blk):
                bc0 = 527 + blk * 21
                ln_g = cp[:, bc0 + 17 : bc0 + 18]
                ln_b = cp[:, bc0 + 18 : bc0 + 19]
                pw_wT = cp[:, bc0 + 19 : bc0 + 21]
                acc = acc_ss[blk]

                # ---------- layernorm over channels ----------
                sq = sb3.tile([128, NS], f32, tag="sq")
                nc.vector.tensor_tensor(out=sq[:, :], in0=acc[:, :], in1=acc[:, :], op=Alu.mult)
                mu_p = tps.tile([1, NS], f32, tag="tl")
                nc.tensor.matmul(out=mu_p[:, :], lhsT=ones128_div, rhs=acc[:, :], start=True, stop=True)
                e2_p = tps.tile([1, NS], f32, tag="tl")
                nc.tensor.matmul(out=e2_p[:, :], lhsT=ones128_div, rhs=sq[:, :], start=True, stop=True)
                stats = sb3.tile([1, 2 * NS], f32, tag="stats")
                nc.vector.tensor_copy(out=stats[:, 0:NS], in_=mu_p[:, :])
                mu2 = sb3.tile([1, NS], f32, tag="mu2")
                nc.vector.tensor_tensor(out=mu2[:, :], in0=stats[:, 0:NS], in1=stats[:, 0:NS], op=Alu.mult)
                var = sb3.tile([1, NS], f32, tag="var")
                nc.vector.tensor_tensor(out=var[:, :], in0=e2_p[:, :], in1=mu2[:, :], op=Alu.subtract)
                sd = sb3.tile([1, NS], f32, tag="sd")
                nc.scalar.activation(out=sd[:, :], in_=var[:, :], func=Act.Sqrt, bias=epst[:, :])
                nc.vector.reciprocal(out=stats[:, NS : 2 * NS], in_=sd[:, :])
                bc_p = tps.tile([128, 2 * NS], f32, tag="tl")
                nc.tensor.matmul(out=bc_p[:, :], lhsT=ones1_128, rhs=stats[:, :], start=True, stop=True)
                t1 = sb3.tile([128, NS], f32, tag="t1")
                nc.vector.tensor_tensor(out=t1[:, :], in0=acc[:, :], in1=bc_p[:, 0:NS], op=Alu.subtract)
                nc.vector.tensor_tensor(out=t1[:, :], in0=t1[:, :], in1=bc_p[:, NS : 2 * NS], op=Alu.mult)
                nc.vector.tensor_scalar(
                    out=t1[:, :], in0=t1[:, :], scalar1=ln_g, scalar2=ln_b,
                    op0=Alu.mult, op1=Alu.add,
                )
                # exact GELU via Abramowitz-Stegun erf (|err| <= 1.5e-7)
                ze = sb3.tile([128, NS], f32, tag="ze")
                nc.scalar.activation(out=ze[:, :], in_=t1[:, :], func=Act.Abs,
                                     bias=zb[:, :], scale=0.7071067811865476)
                tt_ = sb3.tile([128, NS], f32, tag="tt")
                nc.vector.tensor_scalar(out=tt_[:, :], in0=ze[:, :], scalar1=0.3275911,
                                        scalar2=1.0, op0=Alu.mult, op1=Alu.add)
                nc.vector.reciprocal(out=tt_[:, :], in_=tt_[:, :])
                poly = sb3.tile([128, NS], f32, tag="poly")
                A = (1.061405429, -1.453152027, 1.421413741, -0.284496736, 0.254829592)
                nc.vector.tensor_scalar(out=poly[:, :], in0=tt_[:, :], scalar1=A[0],
                                        scalar2=A[1], op0=Alu.mult, op1=Alu.add)
                for a_c in A[2:]:
                    nc.vector.tensor_tensor(out=poly[:, :], in0=poly[:, :], in1=tt_[:, :], op=Alu.mult)
                    nc.vector.tensor_scalar(out=poly[:, :], in0=poly[:, :], scalar1=a_c,
                                            scalar2=None, op0=Alu.add)
                nc.vector.tensor_tensor(out=poly[:, :], in0=poly[:, :], in1=tt_[:, :], op=Alu.mult)
                ez = sb3.tile([128, NS], f32, tag="sq")
                nc.vector.tensor_tensor(out=ez[:, :], in0=ze[:, :], in1=ze[:, :], op=Alu.mult)
                nc.scalar.activation(out=ez[:, :], in_=ez[:, :], func=Act.Exp,
                                     bias=zb[:, :], scale=-1.0)
                nc.vector.tensor_tensor(out=poly[:, :], in0=poly[:, :], in1=ez[:, :], op=Alu.mult)
                # erf_abs = 1 - poly
                nc.vector.tensor_scalar(out=poly[:, :], in0=poly[:, :], scalar1=-1.0,
                                        scalar2=1.0, op0=Alu.mult, op1=Alu.add)
                # phi = 0.5 + sign(x)*0.5*erf_abs ; gelu = x*phi
                nc.vector.tensor_tensor(out=ze[:, :], in0=t1[:, :],
                                        in1=zb[:, :].to_broadcast([128, NS]), op=Alu.is_gt)
                nc.vector.tensor_scalar(out=ze[:, :], in0=ze[:, :], scalar1=1.0,
                                        scalar2=-0.5, op0=Alu.mult, op1=Alu.add)
                nc.vector.tensor_tensor(out=poly[:, :], in0=poly[:, :], in1=ze[:, :], op=Alu.mult)
                nc.vector.tensor_scalar(out=poly[:, :], in0=poly[:, :], scalar1=0.5,
                                        scalar2=None, op0=Alu.add)
                gl = sb3.tile([128, NS], f32, tag="tt")
                nc.vector.tensor_tensor(out=gl[:, :], in0=t1[:, :], in1=poly[:, :], op=Alu.mult)

                # ---------- offsets -> positions ----------
                off_p = tps.tile([2, NS], f32, tag="tl")
                nc.tensor.matmul(out=off_p[:, :], lhsT=pw_wT, rhs=gl[:, :], start=True, stop=True)
                pos = sb3.tile([2, NS], f32, tag="pos")
                nc.vector.tensor_tensor(out=pos[:, :], in0=off_p[:, :], in1=ref_yx, op=Alu.add)
                nc.vector.tensor_scalar(
                    out=pos[:, :], in0=pos[:, :], scalar1=1.0, scalar2=-1.0,
                    op0=Alu.min, op1=Alu.max,
                )

                # transpose pos -> [n,(y,x)] per 128-chunk
                posT = sb3.tile([128, 4], f32, tag="posT")  # cols: c0y c0x c1y c1x
                for c in range(2):
                    tp = tps.tile([128, 2], f32, tag="tl")
                    nc.tensor.transpose(
                        out=tp[:, :], in_=pos[:, c * 128 : (c + 1) * 128], identity=eye[0:2, 0:2]
                    )
                    nc.vector.tensor_copy(out=posT[:, c * 2 : c * 2 + 2], in_=tp[:, :])

                # ---------- per-chunk index & weight math ----------
                idxkv = sb3.tile([128, 8], f32, tag="idxkv")
                idxw = sb3.tile([128, 8], f32, tag="idxw")
                fyb = sb3.tile([128, 2], f32, tag="fyb")
                wkv = sbs2.tile([128, 8], f32, tag="wkv")   # w00 w01 w10 w11 per chunk
                dxw = sb3.tile([128, 4], f32, tag="dxw")   # (1-fxb, fxb) per chunk
                dxwf = sb3.tile([128, 4], f32, tag="dxwf")  # dxw * fyb
                scr = sb3.tile([128, 12], f32, tag="scr")

                for c in range(2):
                    y = posT[:, c * 2 : c * 2 + 1]
                    x = posT[:, c * 2 + 1 : c * 2 + 2]
                    # kv pixel coords
                    xf = scr[:, 0:1]
                    yf = scr[:, 1:2]
                    nc.vector.tensor_scalar(out=xf, in0=x, scalar1=1.0, scalar2=31.5, op0=Alu.add, op1=Alu.mult)
                    nc.vector.tensor_scalar(out=yf, in0=y, scalar1=1.0, scalar2=31.5, op0=Alu.add, op1=Alu.mult)
                    xm = scr[:, 2:3]
                    ym = scr[:, 3:4]
                    x0 = scr[:, 4:5]
                    y0 = scr[:, 5:6]
                    # floor via round-to-nearest (+2^23) then subtract (r > x)
                    nc.vector.tensor_scalar(out=x0, in0=xf, scalar1=8388608.0, scalar2=-8388608.0, op0=Alu.add, op1=Alu.add)
                    nc.vector.tensor_tensor(out=xm, in0=x0, in1=xf, op=Alu.is_gt)
                    nc.vector.tensor_tensor(out=x0, in0=x0, in1=xm, op=Alu.subtract)
                    nc.vector.tensor_scalar(out=x0, in0=x0, scalar1=62.0, scalar2=None, op0=Alu.min)
                    nc.vector.tensor_scalar(out=y0, in0=yf, scalar1=8388608.0, scalar2=-8388608.0, op0=Alu.add, op1=Alu.add)
                    nc.vector.tensor_tensor(out=ym, in0=y0, in1=yf, op=Alu.is_gt)
                    nc.vector.tensor_tensor(out=y0, in0=y0, in1=ym, op=Alu.subtract)
                    nc.vector.tensor_scalar(out=y0, in0=y0, scalar1=62.0, scalar2=None, op0=Alu.min)
                    fx = scr[:, 6:7]
                    fy = scr[:, 7:8]
                    nc.vector.tensor_tensor(out=fx, in0=xf, in1=x0, op=Alu.subtract)
                    nc.vector.tensor_tensor(out=fy, in0=yf, in1=y0, op=Alu.subtract)
                    fx1 = scr[:, 8:9]
                    fy1 = scr[:, 9:10]
                    nc.vector.tensor_scalar(out=fx1, in0=fx, scalar1=-1.0, scalar2=1.0, op0=Alu.mult, op1=Alu.add)
                    nc.vector.tensor_scalar(out=fy1, in0=fy, scalar1=-1.0, scalar2=1.0, op0=Alu.mult, op1=Alu.add)
                    nc.vector.tensor_tensor(out=wkv[:, c * 4 + 0 : c * 4 + 1], in0=fy1, in1=fx1, op=Alu.mult)
                    nc.vector.tensor_tensor(out=wkv[:, c * 4 + 1 : c * 4 + 2], in0=fy1, in1=fx, op=Alu.mult)
                    nc.vector.tensor_tensor(out=wkv[:, c * 4 + 2 : c * 4 + 3], in0=fy, in1=fx1, op=Alu.mult)
                    nc.vector.tensor_tensor(out=wkv[:, c * 4 + 3 : c * 4 + 4], in0=fy, in1=fx, op=Alu.mult)
                    # kv gather indices: y0*64+x0 (+0,+1,+64,+65)
                    ib = scr[:, 10:11]
                    nc.vector.scalar_tensor_tensor(out=ib, in0=y0, scalar=64.0, in1=x0, op0=Alu.mult, op1=Alu.add)
                    for t, offt in enumerate((0.0, 1.0, 64.0, 65.0)):
                        nc.vector.tensor_scalar(
                            out=idxkv[:, c * 4 + t : c * 4 + t + 1], in0=ib,
                            scalar1=offt, scalar2=None, op0=Alu.add,
                        )
                    # bias window coords: cx = 31.5*(1-x), cy = 31.5*(1-y)
                    cxf = scr[:, 0:1]
                    cyf = scr[:, 1:2]
                    nc.vector.tensor_scalar(out=cxf, in0=x, scalar1=-31.5, scalar2=31.5, op0=Alu.mult, op1=Alu.add)
                    nc.vector.tensor_scalar(out=cyf, in0=y, scalar1=-31.5, scalar2=31.5, op0=Alu.mult, op1=Alu.add)
                    fbx = scr[:, 2:3]
                    fby = scr[:, 3:4]
                    x0b = scr[:, 4:5]
                    y0b = scr[:, 5:6]
                    nc.vector.tensor_scalar(out=x0b, in0=cxf, scalar1=8388608.0, scalar2=-8388608.0, op0=Alu.add, op1=Alu.add)
                    nc.vector.tensor_tensor(out=fbx, in0=x0b, in1=cxf, op=Alu.is_gt)
                    nc.vector.tensor_tensor(out=x0b, in0=x0b, in1=fbx, op=Alu.subtract)
                    nc.vector.tensor_scalar(out=y0b, in0=cyf, scalar1=8388608.0, scalar2=-8388608.0, op0=Alu.add, op1=Alu.add)
                    nc.vector.tensor_tensor(out=fby, in0=y0b, in1=cyf, op=Alu.is_gt)
                    nc.vector.tensor_tensor(out=y0b, in0=y0b, in1=fby, op=Alu.subtract)
                    nc.vector.tensor_tensor(out=fbx, in0=cxf, in1=x0b, op=Alu.subtract)
                    nc.vector.tensor_tensor(out=fby, in0=cyf, in1=y0b, op=Alu.subtract)
                    nc.vector.tensor_copy(out=fyb[:, c : c + 1], in_=fby)
                    nc.vector.tensor_scalar(out=dxw[:, c * 2 : c * 2 + 1], in0=fbx, scalar1=-1.0, scalar2=1.0, op0=Alu.mult, op1=Alu.add)
                    nc.vector.tensor_copy(out=dxw[:, c * 2 + 1 : c * 2 + 2], in_=fbx)
                    nc.vector.tensor_tensor(out=dxwf[:, c * 2 : c * 2 + 1], in0=dxw[:, c * 2 : c * 2 + 1], in1=fyb[:, c : c + 1], op=Alu.mult)
                    nc.vector.tensor_tensor(out=dxwf[:, c * 2 + 1 : c * 2 + 2], in0=dxw[:, c * 2 + 1 : c * 2 + 2], in1=fyb[:, c : c + 1], op=Alu.mult)
                    # window index: ((x0b*128)+y0b)*65 + blk_base (+h stride)
                    iw = scr[:, 11:12]
                    nc.vector.scalar_tensor_tensor(out=iw, in0=x0b, scalar=128.0, in1=y0b, op0=Alu.mult, op1=Alu.add)
                    nc.vector.tensor_scalar(
                        out=iw, in0=iw, scalar1=65.0, scalar2=float(blk * TBLK),
                        op0=Alu.mult, op1=Alu.add,
                    )
                    for hh in range(4):
                        nc.vector.tensor_scalar(
                            out=idxw[:, c * 4 + hh : c * 4 + hh + 1], in0=iw,
                            scalar1=float(hh * THEAD), scalar2=None, op0=Alu.add,
                        )

                idxkv_i = sbs2.tile([128, 8], i32, tag="idxkvi")
                nc.vector.tensor_copy(out=idxkv_i[:, :], in_=idxkv[:, :])
                idxw_i = sbs2.tile([128, 8], i32, tag="idxwi")
                nc.vector.tensor_copy(out=idxw_i[:, :], in_=idxw[:, :])

                # diag pairs for DoubleRow bias matmuls, per chunk:
                # dp0[p,0,m]=dxw0[p]*eye, dp0[p,1,m]=dxw0[p]*fyb[p]*eye (x-tap 0)
                # dp1 same with dxw1 (x-tap 1)
                dps = []
                for c in range(2):
                    d0 = sbs2.tile([128, 2, 128], fp8, tag=f"dp0_{c}")
                    d1 = sbs2.tile([128, 2, 128], fp8, tag=f"dp1_{c}")
                    nc.vector.tensor_scalar(out=d0[:, 0, :], in0=eye, scalar1=dxw[:, c * 2 : c * 2 + 1], scalar2=None, op0=Alu.mult)
                    nc.vector.tensor_scalar(out=d0[:, 1, :], in0=eye, scalar1=dxwf[:, c * 2 : c * 2 + 1], scalar2=None, op0=Alu.mult)
                    nc.vector.tensor_scalar(out=d1[:, 0, :], in0=eye, scalar1=dxw[:, c * 2 + 1 : c * 2 + 2], scalar2=None, op0=Alu.mult)
                    nc.vector.tensor_scalar(out=d1[:, 1, :], in0=eye, scalar1=dxwf[:, c * 2 + 1 : c * 2 + 2], scalar2=None, op0=Alu.mult)
                    dps.append((d0, d1))

                return dict(idxw_i=idxw_i, dps=dps, wkv=wkv, idxkv_i=idxkv_i)

            def emit_kv(blk, st):
                wkv, idxkv_i = st["wkv"], st["idxkv_i"]
                # ---------- kv gather + k/v projections ----------
                kvT_ap = BLKS[blk][1]
                pk_wTs1 = wb(blk, 0, 128, rows=65)
                pv_wT1 = wb(blk, 128, 192, rows=65)
                G = sb3.tile([128, 8, 64], f32, tag="G")
                for j in range(8):
                    nc.gpsimd.indirect_dma_start(
                        out=G[:, j, :], out_offset=None, in_=kvT_ap,
                        in_offset=IndirectOffsetOnAxis(ap=idxkv_i[:, j : j + 1], axis=0),
                    )
                xs_b = sb3.tile([65, NS], bf16, tag="xsb")
                nc.vector.memset(xs_b[64:65, :], 1.0)
                for c in range(2):
                    xsT = sb3.tile([128, 64], f32, tag="xsT")
                    nc.vector.tensor_scalar(
                        out=xsT[:, :], in0=G[:, c * 4 + 0, :],
                        scalar1=wkv[:, c * 4 : c * 4 + 1], scalar2=None, op0=Alu.mult,
                    )
                    for t in range(1, 4):
                        nc.vector.scalar_tensor_tensor(
                            out=xsT[:, :], in0=G[:, c * 4 + t, :],
                            scalar=wkv[:, c * 4 + t : c * 4 + t + 1], in1=xsT[:, :],
                            op0=Alu.mult, op1=Alu.add,
                        )
                    xs_p = tps.tile([64, 128], f32, tag="tl")
                    nc.tensor.transpose(out=xs_p[:, :], in_=xsT[:, :], identity=eye)
                    nc.vector.tensor_copy(out=xs_b[0:64, c * 128 : (c + 1) * 128], in_=xs_p[:, :])

                k_p = tps.tile([128, NS], f32, tag="tl")
                nc.tensor.matmul(out=k_p[:, :], lhsT=pk_wTs1, rhs=xs_b[:, :], start=True, stop=True)
                # per-head masked k: [128, c, h, 128] with only rows h*32..+16 nonzero
                k_b4 = sbs2.tile([128, 2, 4, 128], bf16, tag="kb4")
                nc.gpsimd.memset(k_b4[:, :, :, :], 0.0)
                for c in range(2):
                    for h in range(4):
                        nc.vector.tensor_copy(
                            out=k_b4[h * 32 : h * 32 + 16, c, h, :],
                            in_=k_p[h * 32 : h * 32 + 16, c * 128 : (c + 1) * 128],
                        )

                vT1 = sbs2.tile([128, 2, 128], bf16, tag="vT1")
                nc.vector.memset(vT1[:, :, :], 0.0)
                nc.vector.memset(vT1[:, :, :].rearrange("p c (h q) -> p c h q", q=32)[:, :, :, 16:17], 1.0)
                for c in range(2):
                    v_p = tps.tile([128, 64], f32, tag="tl")
                    nc.tensor.matmul(
                        out=v_p[:, :], lhsT=xs_b[:, c * 128 : (c + 1) * 128], rhs=pv_wT1,
                        start=True, stop=True,
                    )
                    vv = vT1[:, c, :].rearrange("p (h q) -> p h q", q=32)
                    nc.vector.tensor_copy(
                        out=vv[:, :, 0:16],
                        in_=v_p[:, :].rearrange("p (h q) -> p h q", q=16),
                    )
                st["k_b4"] = k_b4
                st["vT1"] = vT1

            # ======= attention QK+bias+exp for one head-pair of a block =====
            def emit_qk_pair(blk, st, pg):
                q_b = q_bs[blk]
                idxw_i, dps, k_b4 = st["idxw_i"], st["dps"], st["k_b4"]
                P = ppool.tile([128, 2, 2, HWS], bf16, tag="P")
                st["P"] = P
                for hp in range(2):
                    h = pg * 2 + hp
                    for c in range(2):
                        G2 = gpool.tile([128, 2, 4160], fp8, tag="g2")
                        nc.gpsimd.indirect_dma_start(
                            out=G2[:, 0, :], out_offset=None, in_=tab_d,
                            in_offset=IndirectOffsetOnAxis(ap=idxw_i[:, c * 4 + h : c * 4 + h + 1], axis=0),
                        )
                        nc.gpsimd.indirect_dma_start(
                            out=G2[:, 1, :], out_offset=None, in_=tab_d,
                            in_offset=IndirectOffsetOnAxis(ap=idxw_i[:, c * 4 + h : c * 4 + h + 1], axis=0),
                            element_offset=NTAB,
                        )
                        G2r = G2[:, :, :].rearrange("p t (r q) -> p t r q", q=65)
                        d0, d1 = dps[c]
                        kh = k_b4[:, c, h, :]
                        for wv in range(2):
                            ptds = []
                            for j in range(2):
                                ptd = qkps.tile([128, 1024], f32, tag="qkp")
                                ptds.append(ptd)
                                for half in range(2):
                                    mc = wv * 4 + j * 2 + half
                                    pt = ptd[:, half * 512 : (half + 1) * 512]
                                    nc.tensor.matmul(
                                        out=pt, lhsT=kh,
                                        rhs=q_b[:, mc * 512 : (mc + 1) * 512],
                                        start=True, stop=False,
                                        skip_group_check=True,
                                    )
                                    nc.tensor.matmul(
                                        out=pt, lhsT=d0[:, :, :],
                                        rhs=G2r[:, :, mc * 8 : (mc + 1) * 8, 0:64],
                                        start=False, stop=False, perf_mode=DR,
                                        skip_group_check=True,
                                    )
                                    nc.tensor.matmul(
                                        out=pt, lhsT=d1[:, :, :],
                                        rhs=G2r[:, :, mc * 8 : (mc + 1) * 8, 1:65],
                                        start=False, stop=True, perf_mode=DR,
                                        skip_group_check=True,
                                    )
                            for j in range(2):
                                base = (wv * 4 + j * 2) * 512
                                nc.scalar.activation(
                                    out=P[:, hp, c, base : base + 1024], in_=ptds[j][:, :],
                                    func=Act.Exp, bias=zb[:, :],
                                )

            # =================== AV for one head-pair =======================
            def emit_avpair(blk, st, pg):
                P, vT1 = st["P"], st["vT1"]
                avs = st["avs"]
                for mc in range(8):
                    av = avps.tile([64, 512], f32, tag="avp")
                    for hp in range(2):
                        h = pg * 2 + hp
                        for c in range(2):
                            nc.tensor.matmul(
                                out=av[hp * 32 : (hp + 1) * 32, :],
                                lhsT=vT1[:, c, h * 32 : (h + 1) * 32],
                                rhs=P[:, hp, c, mc * 512 : (mc + 1) * 512],
                                start=(c == 0), stop=(c == 1),
                                skip_group_check=True,
                                tile_position=(0, hp * 32),
                            )
                    dst = avs[pg * 64 : (pg + 1) * 64, mc * 512 : (mc + 1) * 512]
                    nc.vector.tensor_copy(out=dst, in_=av[:, :])

            # ========================== tail ================================
            def emit_tail(blk, st):
                avs = st["avs"]
                R = BLKS[blk][2]
                po_wT_sp = cpb[:, 128 + blk * 64 : 128 + (blk + 1) * 64]
                b4 = cpb[:, 0:128]
                po_b_hi = cp[64:128, 524 + blk : 525 + blk]
                rcp_all = sbt.tile([128, HWS], bf16, tag="rcpa", bufs=1)
                for mc in range(8):
                    sb_p = tps.tile([128, 512], f32, tag="tl")
                    nc.tensor.matmul(out=sb_p[:, :], lhsT=b4, rhs=avs[:, mc * 512 : (mc + 1) * 512], start=True, stop=True)
                    act_raw(rcp_all[:, mc * 512 : (mc + 1) * 512], sb_p[:, :], Act.Reciprocal)
                for mc in range(8):
                    on = sbt.tile([128, 512], bf16, tag="on", bufs=1)
                    nc.vector.tensor_tensor(out=on[:, :], in0=avs[:, mc * 512 : (mc + 1) * 512], in1=rcp_all[:, mc * 512 : (mc + 1) * 512], op=Alu.mult)
                    op = tps.tile([64, 512], f32, tag="tl")
                    nc.tensor.matmul(out=op[:, :], lhsT=po_wT_sp, rhs=on[:, :], start=True, stop=True)
                    nc.vector.scalar_tensor_tensor(
                        out=R[64:128, mc * 512 : (mc + 1) * 512], in0=op[:, :], scalar=po_b_hi,
                        in1=R[64:128, mc * 512 : (mc + 1) * 512], op0=Alu.add, op1=Alu.add,
                    )

            # ========================= main schedule ========================
            emit_qproj(0)
            emit_qproj(1)
            emit_conv(0)
            emit_qproj(2)
            emit_conv(1)
            emit_conv(2)

            def emit_block_attn(blk, st):
                st["avs"] = apool.tile([128, HWS], bf16, tag="avs", name="avs")
                emit_qk_pair(blk, st, 0)
                emit_avpair(blk, st, 0)
                emit_qk_pair(blk, st, 1)
                emit_avpair(blk, st, 1)

            nc.sync.dma_start(out=o1_d[0:64, :], in_=xi1[0:64, :])
            nc.sync.dma_start(out=o2_d[0:64, :], in_=xi2[0:64, :])
            st0 = emit_prologue(0)
            st1 = emit_prologue(1)
            st2 = emit_prologue(2)
            emit_kv(0, st0)
            emit_block_attn(0, st0)
            emit_kv(1, st1)
            emit_tail(0, st0)
            nc.sync.dma_start(out=o1_d[64:128, :], in_=xi1[64:128, :])
            emit_block_attn(1, st1)
            emit_kv(2, st2)
            emit_tail(1, st1)
            emit_block_attn(2, st2)
            emit_tail(2, st2)

            nc.sync.dma_start(out=o2_d[64:128, :], in_=xi2[64:128, :])

    nc.compile()
    return nc


def _host_prep(inputs):
    """Build per-core in_maps. inputs: dict of full numpy arrays."""
    import ml_dtypes

    x0, x1, x2 = inputs["x0"], inputs["x1"], inputs["x2"]

    def spread_cols(m):
        # m: [64(in), 64(out)] -> [64(in), 128] with out col h*16+j at h*32+j
        out = np.zeros((m.shape[0], 128), m.dtype)
        for h in range(4):
            out[:, h * 32 : h * 32 + 16] = m[:, h * 16 : (h + 1) * 16]
        return out

    def spread_rows(v):
        # v: [64, k] -> [128, k] with row h*16+j at h*32+j
        out = np.zeros((128,) + v.shape[1:], v.dtype)
        for h in range(4):
            out[h * 32 : h * 32 + 16] = v[h * 16 : (h + 1) * 16]
        return out

    # weight pack f32 (used as f32r): [64, 3*128]  (spread pq_wT)
    wpf = np.zeros((64, 3 * 128), np.float32)
    for b in range(3):
        wpf[:, b * 128 : (b + 1) * 128] = spread_cols(inputs["pq_w"][b].T)
    wpb = np.zeros((65, 3 * 192), ml_dtypes.bfloat16)
    for b in range(3):
        o = b * 192
        pk = np.zeros((65, 128), np.float32)
        pk[0:64] = spread_cols(inputs["pk_w"][b].T * 0.25)
        for h in range(4):
            pk[64, h * 32 : h * 32 + 16] = inputs["pk_b"][b][h * 16 : (h + 1) * 16] * 0.25
        wpb[:, o : o + 128] = pk.astype(ml_dtypes.bfloat16)
        wpb[:64, o + 128 : o + 192] = inputs["pv_w"][b].T.astype(ml_dtypes.bfloat16)
        wpb[64, o + 128 : o + 192] = inputs["pv_b"][b].astype(ml_dtypes.bfloat16)
    # const pack [128, 590]
    cp = np.zeros((128, 590), np.float32)
    cp[:, 0:128] = np.eye(128, dtype=np.float32)
    ys = (np.linspace(0.5, HK - 0.5, HK) / (HK - 1.0)) * 2.0 - 1.0
    cp[0, 128:384] = np.repeat(ys, WK)         # y per n (i-major)
    cp[1, 128:384] = np.tile(ys, HK)           # x per n
    cp[0, 384:512] = 1.0                       # ones1_128
    for h in range(4):
        cp[h * 32 : h * 32 + 16, 520] = 1.0 / 64.0
    for b in range(3):
        cp[:, 521 + b] = spread_rows(inputs["pq_b"][b][:, None])[:, 0]
        cp[64:128, 524 + b] = inputs["po_b"][b]
        bc0 = 527 + b * 21
        cp[:, bc0 : bc0 + 16] = spread_rows(inputs["dw_w"][b].reshape(64, 16))
        cp[:, bc0 + 16] = spread_rows(inputs["dw_b"][b][:, None])[:, 0]
        cp[:, bc0 + 17] = spread_rows(inputs["ln_g"][b][:, None])[:, 0]
        cp[:, bc0 + 18] = spread_rows(inputs["ln_b"][b][:, None])[:, 0]
        cp[:, bc0 + 19 : bc0 + 21] = spread_rows(inputs["pw_w"][b].T)
    cpb = np.zeros((128, 320), ml_dtypes.bfloat16)
    b4 = np.zeros((128, 128), np.float32)
    for h in range(4):
        b4[h * 32 + 16, h * 32 : (h + 1) * 32] = 1.0
    cpb[:, 0:128] = b4.astype(ml_dtypes.bfloat16)
    for b in range(3):
        poT = inputs["po_w"][b].T  # [c, o]
        for h in range(4):
            cpb[h * 32 : h * 32 + 16, 128 + b * 64 : 128 + (b + 1) * 64] = poT[
                h * 16 : (h + 1) * 16
            ].astype(ml_dtypes.bfloat16)
    # depthwise conv diag consts bf16 (hi) + bf16 residual (lo)
    cdg = np.zeros((128, 3 * 16 * 128), ml_dtypes.bfloat16)
    cdgl = np.zeros((128, 3 * 16 * 128), ml_dtypes.bfloat16)
    for b in range(3):
        wsp = spread_rows(inputs["dw_w"][b].reshape(64, 16))  # [128, 16]
        whi = wsp.astype(ml_dtypes.bfloat16).astype(np.float32)
        wlo = wsp - whi
        for t in range(16):
            d = np.zeros((128, 128), np.float32)
            np.fill_diagonal(d, whi[:, t])
            cdg[:, (b * 16 + t) * 128 : (b * 16 + t + 1) * 128] = d.astype(ml_dtypes.bfloat16)
            np.fill_diagonal(d, wlo[:, t])
            cdgl[:, (b * 16 + t) * 128 : (b * 16 + t + 1) * 128] = d.astype(ml_dtypes.bfloat16)
    # rpe slice tables fp8: T windows then D (row-diff) windows
    tab = np.zeros((2, NBLK, NH, 64, TROW, TCOL), ml_dtypes.float8_e4m3)
    rpe = inputs["rpe"]
    for b in range(3):
        for h in range(4):
            pad = np.zeros((129, 128), np.float32)
            pad[0:127, 0:127] = rpe[b, h]
            dif = pad[1:129] - pad[0:128]
            for x0s in range(64):
                tab[0, b, h, x0s] = pad[0:128, x0s : x0s + 65].astype(ml_dtypes.float8_e4m3)
                tab[1, b, h, x0s] = dif[:, x0s : x0s + 65].astype(ml_dtypes.float8_e4m3)
    tab = tab.reshape(-1, 1)

    in_maps = []
    for bb in range(B):
        m = {
            "xi1": np.ascontiguousarray(x1[bb].reshape(C, HWS)),
            "xi2": np.ascontiguousarray(x2[bb].reshape(C, HWS)),
            "xq1": np.ascontiguousarray(x1[bb, :64].reshape(64, HWS)),
            "xq2": np.ascontiguousarray(x2[bb, :64].reshape(64, HWS)),
            "kvT0": np.ascontiguousarray(x0[bb, :64].reshape(64, HWS).T),
            "kvT1": np.ascontiguousarray(x1[bb, :64].reshape(64, HWS).T),
            "wpf": wpf,
            "wpb": wpb,
            "cp": cp,
            "cpb": cpb,
            "cdg": cdg,
            "cdgl": cdgl,
            "rpetab": tab,
        }
        in_maps.append(m)
    return in_maps


def kernel(**inputs):
    from concourse.bass_utils import run_bass_kernel_spmd

    if "nc" not in _CACHE:
        _CACHE["nc"] = _build_graph()
    nc = _CACHE["nc"]
    in_maps = _host_prep(inputs)
    res = run_bass_kernel_spmd(nc, in_maps, core_ids=list(range(8)))
    out = np.zeros((NBLK, B, C, H, W), np.float32)
    out[0] = inputs["x0"]
    for bb in range(B):
        out[1, bb] = res.results[bb]["o1"].reshape(C, H, W)
        out[2, bb] = res.results[bb]["o2"].reshape(C, H, W)
    return out


# revision 21
# speedup vs baseline: 1.0490x; 1.0490x over previous
"""Trainium2 Bass kernel for nn_AttentionTD (3-block deformable attention TD).

Self-contained: hardcodes all shapes. Data-parallel over batch B=8 across the
8 NeuronCores; each core runs the full 3-block DAT stack for one batch element.

v3: precise offset path (f32r qproj + hi/lo bf16 depthwise conv), fp8
DoubleRow bias matmuls folding the fy-interp into the PE, head-pair packed AV
psum, 1024-wide exp chunks, scalar-engine reciprocal, phase-interleaved
emission across the three independent blocks.
"""

import sys

sys.path.insert(0, "/opt/trn_rl_repo")

import numpy as np

# ---------------- problem constants ----------------
B, C, H, W = 8, 128, 64, 64
NCH = 64          # channels per DAT block
NH, HC = 4, 16    # heads, head channels
KS = 4
HWS = H * W       # 4096
HK = WK = 16
NS = HK * WK      # 256 sample points
EPS = 1e-5
NBLK = 3
# rpe slice table geometry: [blk][h][x0 (64)][row (128)][col (65)]
TROW, TCOL = 128, 65
TSLICE = TROW * TCOL          # 8320
THEAD = 64 * TSLICE           # per (blk,h)
TBLK = NH * THEAD
NTAB = NBLK * TBLK

_CACHE = {}


def _build_graph():
    from concourse import bacc, mybir, tile
    import concourse.bass as bass
    from concourse.bass import IndirectOffsetOnAxis

    f32 = mybir.dt.float32
    f32r = mybir.dt.float32r
    bf16 = mybir.dt.bfloat16
    fp8 = mybir.dt.float8e4
    i32 = mybir.dt.int32
    Alu = mybir.AluOpType
    Act = mybir.ActivationFunctionType
    DR = mybir.MatmulPerfMode.DoubleRow

    nc = bacc.Bacc("TRN2", target_bir_lowering=False, debug=False, num_devices=8)

    # ---- dram io ----
    xi1_d = nc.dram_tensor("xi1", [C, HWS], f32, kind="ExternalInput").ap()
    xi2_d = nc.dram_tensor("xi2", [C, HWS], f32, kind="ExternalInput").ap()
    xq1_d = nc.dram_tensor("xq1", [64, HWS], f32r, kind="ExternalInput").ap()
    xq2_d = nc.dram_tensor("xq2", [64, HWS], f32r, kind="ExternalInput").ap()
    kvT0_d = nc.dram_tensor("kvT0", [HWS, NCH], f32, kind="ExternalInput").ap()
    kvT1_d = nc.dram_tensor("kvT1", [HWS, NCH], f32, kind="ExternalInput").ap()
    wpf_d = nc.dram_tensor("wpf", [64, 3 * 128], f32r, kind="ExternalInput").ap()
    wpb_d = nc.dram_tensor("wpb", [65, 3 * 192], bf16, kind="ExternalInput").ap()
    cp_d = nc.dram_tensor("cp", [128, 590], f32, kind="ExternalInput").ap()
    cpb_d = nc.dram_tensor("cpb", [128, 320], bf16, kind="ExternalInput").ap()
    cdg_d = nc.dram_tensor("cdg", [128, 3 * 16 * 128], bf16, kind="ExternalInput").ap()
    cdgl_d = nc.dram_tensor("cdgl", [128, 3 * 16 * 128], bf16, kind="ExternalInput").ap()
    tab_d = nc.dram_tensor("rpetab", [2 * NTAB, 1], fp8, kind="ExternalInput").ap()
    o1_d = nc.dram_tensor("o1", [C, HWS], f32, kind="ExternalOutput").ap()
    o2_d = nc.dram_tensor("o2", [C, HWS], f32, kind="ExternalOutput").ap()

    with tile.TileContext(nc) as tc:
        import contextlib

        ctx = contextlib.ExitStack()
        with ctx:
            cpool = ctx.enter_context(tc.tile_pool(name="const", bufs=1))
            xpool = ctx.enter_context(tc.tile_pool(name="xdata", bufs=1))
            qpool = ctx.enter_context(tc.tile_pool(name="qtiles", bufs=3))
            lpool = ctx.enter_context(tc.tile_pool(name="qlo", bufs=2))
            spool = ctx.enter_context(tc.tile_pool(name="stage", bufs=2))
            ppool = ctx.enter_context(tc.tile_pool(name="probs", bufs=1))
            apool = ctx.enter_context(tc.tile_pool(name="avs", bufs=2))
            gpool = ctx.enter_context(tc.tile_pool(name="wins", bufs=2))
            sb3 = ctx.enter_context(tc.tile_pool(name="blk", bufs=1))
            sbs2 = ctx.enter_context(tc.tile_pool(name="blkstate", bufs=2))
            sba = ctx.enter_context(tc.tile_pool(name="accs", bufs=3))
            sbt = ctx.enter_context(tc.tile_pool(name="tails", bufs=2))
            qkps = ctx.enter_context(tc.tile_pool(name="qk", bufs=2, space="PSUM"))
            avps = ctx.enter_context(tc.tile_pool(name="av", bufs=2, space="PSUM"))
            tps = ctx.enter_context(tc.tile_pool(name="tailp", bufs=2, space="PSUM"))

            # ---- persistent loads ----
            cp = cpool.tile([128, 590], f32, tag="cp")
            nc.sync.dma_start(out=cp[:, :], in_=cp_d)
            wpf = cpool.tile([64, 3 * 128], f32r, tag="wpf")
            nc.sync.dma_start(out=wpf[:, :], in_=wpf_d)
            wpb = cpool.tile([65, 3 * 192], bf16, tag="wpb")
            cpb = cpool.tile([128, 320], bf16, tag="cpb")
            cdg = cpool.tile([128, 3 * 16 * 128], bf16, tag="cdg")
            cdgl = cpool.tile([128, 3 * 16 * 128], bf16, tag="cdgl")
            xi1 = xpool.tile([C, HWS], f32, tag="xi1")
            xi2 = xpool.tile([C, HWS], f32, tag="xi2")

            zb = cpool.tile([128, 1], f32, tag="zb")
            nc.vector.memset(zb[:, :], 0.0)
            epst = cpool.tile([1, 1], f32, tag="epst")
            nc.vector.memset(epst[:, :], EPS)

            eye = cp[:, 0:128]
            ref_yx = cp[0:2, 128:384]          # row0 = y, row1 = x
            ones1_128 = cp[0:1, 384:512]       # [1,128] ones (bcast lhsT)
            ones128_div = cp[0:128, 520:521]   # 1/64 on data rows, 0 on gaps

            def act_raw(out, in_, func):
                eng = nc.scalar
                ins = [eng.lower_ap(in_)]
                for v in (0.0, 1.0, 0.0):
                    ins.append(mybir.ImmediateValue(dtype=mybir.dt.float32, value=v))
                return eng.add_instruction(
                    mybir.InstActivation(
                        name=nc.get_next_instruction_name(), func=func,
                        ins=ins, outs=[eng.lower_ap(out)],
                    )
                )

            def wf(blk, lo, hi):
                return wpf[:, blk * 128 + lo : blk * 128 + hi]

            def wb(blk, lo, hi, rows=64):
                return wpb[0:rows, blk * 192 + lo : blk * 192 + hi]

            BLKS = [(xq1_d, kvT0_d, xi1), (xq2_d, kvT0_d, xi2), (xq2_d, kvT1_d, xi2)]

            # ============ Phase A2: q projections (f32r via staging) ========
            q_bs = [None, None, None]
            q_ls = [None, None, None]

            def emit_qproj(blk):
                XQ_d = BLKS[blk][0]
                pq_wT_sp = wf(blk, 0, 128)
                pq_b_sp = cp[:, 521 + blk : 522 + blk]
                q_b = qpool.tile([128, HWS], bf16, tag="qb")
                q_l = lpool.tile([128, HWS], bf16, tag="ql")
                q_bs[blk] = q_b
                q_ls[blk] = q_l
                for mc in range(8):
                    stg = spool.tile([64, 512], f32r, tag="stg")
                    nc.sync.dma_start(out=stg[:, :], in_=XQ_d[:, mc * 512 : (mc + 1) * 512])
                    if mc % 2 == 0:
                        qp = qkps.tile([128, 512], f32, tag="qkp")
                    else:
                        qp = tps.tile([128, 512], f32, tag="tl")
                    nc.tensor.matmul(
                        out=qp[:, :], lhsT=pq_wT_sp, rhs=stg[:, :],
                        start=True, stop=True,
                    )
                    nc.scalar.activation(
                        out=q_b[:, mc * 512 : (mc + 1) * 512], in_=qp[:, :],
                        func=Act.Identity, bias=pq_b_sp,
                    )
                    # q_lo = (psum + bias) - q_b  (bf16 residual)
                    nc.vector.scalar_tensor_tensor(
                        out=q_l[:, mc * 512 : (mc + 1) * 512], in0=qp[:, :],
                        scalar=pq_b_sp, in1=q_b[:, mc * 512 : (mc + 1) * 512],
                        op0=Alu.add, op1=Alu.subtract,
                    )

            # ============ Phase A3: depthwise conv (PE diag, hi/lo) =========
            acc_ss = [None, None, None]

            def emit_conv(blk):
                q_b, q_l = q_bs[blk], q_ls[blk]
                q5 = q_b[:, :].rearrange("p (hh a ww b) -> p hh a ww b", hh=16, a=4, ww=16, b=4)
                q5l = q_l[:, :].rearrange("p (hh a ww b) -> p hh a ww b", hh=16, a=4, ww=16, b=4)
                acc_ps = tps.tile([128, NS], f32, tag="tl")
                first = True
                for grp, (lhs, rhsview) in enumerate(((cdg, q5), (cdgl, q5), (cdg, q5l))):
                    for t in range(16):
                        dy, dx = t // 4, t % 4
                        nc.tensor.matmul(
                            out=acc_ps[:, :],
                            lhsT=lhs[:, (blk * 16 + t) * 128 : (blk * 16 + t + 1) * 128],
                            rhs=rhsview[:, :, dy, :, dx],
                            start=first, stop=(grp == 2 and t == 15),
                        )
                        first = False
                dw_b = cp[:, 527 + blk * 21 + 16 : 527 + blk * 21 + 17]
                acc_s = sba.tile([128, NS], f32, tag="dwacc")
                acc_ss[blk] = acc_s
                nc.vector.tensor_scalar(
                    out=acc_s[:, :], in0=acc_ps[:, :], scalar1=dw_b, scalar2=None, op0=Alu.add
                )

            # ============ per-block prologue (LN, GELU, offsets, idx) =======
            def emit_prologue(blk):
                bc0 = 527 + blk * 21
                ln_g = cp[:, bc0 + 17 : bc0 + 18]
                ln_b = cp[:, bc0 + 18 : bc0 + 19]
                pw_wT = cp[:, bc0 + 19 : bc0 + 21]
                acc = acc_ss[blk]

                # ---------- layernorm over channels ----------
                sq = sb3.tile([128, NS], f32, tag="sq")
                nc.vector.tensor_tensor(out=sq[:, :], in0=acc[:, :], in1=acc[:, :], op=Alu.mult)
                mu_p = tps.tile([1, NS], f32, tag="tl")
                nc.tensor.matmul(out=mu_p[:, :], lhsT=ones128_div, rhs=acc[:, :], start=True, stop=True)
                e2_p = tps.tile([1, NS], f32, tag="tl")
                nc.tensor.matmul(out=e2_p[:, :], lhsT=ones128_div, rhs=sq[:, :], start=True, stop=True)
                stats = sb3.tile([1, 2 * NS], f32, tag="stats")
                nc.vector.tensor_copy(out=stats[:, 0:NS], in_=mu_p[:, :])
                mu2 = sb3.tile([1, NS], f32, tag="mu2")
                nc.vector.tensor_tensor(out=mu2[:, :], in0=stats[:, 0:NS], in1=stats[:, 0:NS], op=Alu.mult)
                var = sb3.tile([1, NS], f32, tag="var")
                nc.vector.tensor_tensor(out=var[:, :], in0=e2_p[:, :], in1=mu2[:, :], op=Alu.subtract)
                sd = sb3.tile([1, NS], f32, tag="sd")
                nc.scalar.activation(out=sd[:, :], in_=var[:, :], func=Act.Sqrt, bias=epst[:, :])
                nc.vector.reciprocal(out=stats[:, NS : 2 * NS], in_=sd[:, :])
                bc_p = tps.tile([128, 2 * NS], f32, tag="tl")
                nc.tensor.matmul(out=bc_p[:, :], lhsT=ones1_128, rhs=stats[:, :], start=True, stop=True)
                t1 = sb3.tile([128, NS], f32, tag="t1")
                nc.vector.tensor_tensor(out=t1[:, :], in0=acc[:, :], in1=bc_p[:, 0:NS], op=Alu.subtract)
                nc.vector.tensor_tensor(out=t1[:, :], in0=t1[:, :], in1=bc_p[:, NS : 2 * NS], op=Alu.mult)
                nc.vector.tensor_scalar(
                    out=t1[:, :], in0=t1[:, :], scalar1=ln_g, scalar2=ln_b,
                    op0=Alu.mult, op1=Alu.add,
                )
                # exact GELU via Abramowitz-Stegun erf (|err| <= 1.5e-7)
                ze = sb3.tile([128, NS], f32, tag="ze")
                nc.scalar.activation(out=ze[:, :], in_=t1[:, :], func=Act.Abs,
                                     bias=zb[:, :], scale=0.7071067811865476)
                tt_ = sb3.tile([128, NS], f32, tag="tt")
                nc.vector.tensor_scalar(out=tt_[:, :], in0=ze[:, :], scalar1=0.3275911,
                                        scalar2=1.0, op0=Alu.mult, op1=Alu.add)
                nc.vector.reciprocal(out=tt_[:, :], in_=tt_[:, :])
                poly = sb3.tile([128, NS], f32, tag="poly")
                A = (1.061405429, -1.453152027, 1.421413741, -0.284496736, 0.254829592)
                nc.vector.tensor_scalar(out=poly[:, :], in0=tt_[:, :], scalar1=A[0],
                                        scalar2=A[1], op0=Alu.mult, op1=Alu.add)
                for a_c in A[2:]:
                    nc.vector.tensor_tensor(out=poly[:, :], in0=poly[:, :], in1=tt_[:, :], op=Alu.mult)
                    nc.vector.tensor_scalar(out=poly[:, :], in0=poly[:, :], scalar1=a_c,
                                            scalar2=None, op0=Alu.add)
                nc.vector.tensor_tensor(out=poly[:, :], in0=poly[:, :], in1=tt_[:, :], op=Alu.mult)
                ez = sb3.tile([128, NS], f32, tag="sq")
                nc.vector.tensor_tensor(out=ez[:, :], in0=ze[:, :], in1=ze[:, :], op=Alu.mult)
                nc.scalar.activation(out=ez[:, :], in_=ez[:, :], func=Act.Exp,
                                     bias=zb[:, :], scale=-1.0)
                nc.vector.tensor_tensor(out=poly[:, :], in0=poly[:, :], in1=ez[:, :], op=Alu.mult)
                # erf_abs = 1 - poly
                nc.vector.tensor_scalar(out=poly[:, :], in0=poly[:, :], scalar1=-1.0,
                                        scalar2=1.0, op0=Alu.mult, op1=Alu.add)
                # phi = 0.5 + sign(x)*0.5*erf_abs ; gelu = x*phi
                nc.vector.tensor_tensor(out=ze[:, :], in0=t1[:, :],
                                        in1=zb[:, :].to_broadcast([128, NS]), op=Alu.is_gt)
                nc.vector.tensor_scalar(out=ze[:, :], in0=ze[:, :], scalar1=1.0,
                                        scalar2=-0.5, op0=Alu.mult, op1=Alu.add)
                nc.vector.tensor_tensor(out=poly[:, :], in0=poly[:, :], in1=ze[:, :], op=Alu.mult)
                nc.vector.tensor_scalar(out=poly[:, :], in0=poly[:, :], scalar1=0.5,
                                        scalar2=None, op0=Alu.add)
                gl = sb3.tile([128, NS], f32, tag="tt")
                nc.vector.tensor_tensor(out=gl[:, :], in0=t1[:, :], in1=poly[:, :], op=Alu.mult)

                # ---------- offsets -> positions ----------
                off_p = tps.tile([2, NS], f32, tag="tl")
                nc.tensor.matmul(out=off_p[:, :], lhsT=pw_wT, rhs=gl[:, :], start=True, stop=True)
                pos = sb3.tile([2, NS], f32, tag="pos")
                nc.vector.tensor_tensor(out=pos[:, :], in0=off_p[:, :], in1=ref_yx, op=Alu.add)
                nc.vector.tensor_scalar(
                    out=pos[:, :], in0=pos[:, :], scalar1=1.0, scalar2=-1.0,
                    op0=Alu.min, op1=Alu.max,
                )

                # transpose pos -> [n,(y,x)] per 128-chunk
                posT = sb3.tile([128, 4], f32, tag="posT")  # cols: c0y c0x c1y c1x
                for c in range(2):
                    tp = tps.tile([128, 2], f32, tag="tl")
                    nc.tensor.transpose(
                        out=tp[:, :], in_=pos[:, c * 128 : (c + 1) * 128], identity=eye[0:2, 0:2]
                    )
                    nc.vector.tensor_copy(out=posT[:, c * 2 : c * 2 + 2], in_=tp[:, :])

                # ---------- per-chunk index & weight math ----------
                idxkv = sb3.tile([128, 8], f32, tag="idxkv")
                idxw = sb3.tile([128, 8], f32, tag="idxw")
                fyb = sb3.tile([128, 2], f32, tag="fyb")
                wkv = sbs2.tile([128, 8], f32, tag="wkv")   # w00 w01 w10 w11 per chunk
                dxw = sb3.tile([128, 4], f32, tag="dxw")   # (1-fxb, fxb) per chunk
                dxwf = sb3.tile([128, 4], f32, tag="dxwf")  # dxw * fyb
                scr = sb3.tile([128, 12], f32, tag="scr")

                for c in range(2):
                    y = posT[:, c * 2 : c * 2 + 1]
                    x = posT[:, c * 2 + 1 : c * 2 + 2]
                    # kv pixel coords
                    xf = scr[:, 0:1]
                    yf = scr[:, 1:2]
                    nc.vector.tensor_scalar(out=xf, in0=x, scalar1=1.0, scalar2=31.5, op0=Alu.add, op1=Alu.mult)
                    nc.vector.tensor_scalar(out=yf, in0=y, scalar1=1.0, scalar2=31.5, op0=Alu.add, op1=Alu.mult)
                    xm = scr[:, 2:3]
                    ym = scr[:, 3:4]
                    x0 = scr[:, 4:5]
                    y0 = scr[:, 5:6]
                    # floor via round-to-nearest (+2^23) then subtract (r > x)
                    nc.vector.tensor_scalar(out=x0, in0=xf, scalar1=8388608.0, scalar2=-8388608.0, op0=Alu.add, op1=Alu.add)
                    nc.vector.tensor_tensor(out=xm, in0=x0, in1=xf, op=Alu.is_gt)
                    nc.vector.tensor_tensor(out=x0, in0=x0, in1=xm, op=Alu.subtract)
                    nc.vector.tensor_scalar(out=x0, in0=x0, scalar1=62.0, scalar2=None, op0=Alu.min)
                    nc.vector.tensor_scalar(out=y0, in0=yf, scalar1=8388608.0, scalar2=-8388608.0, op0=Alu.add, op1=Alu.add)
                    nc.vector.tensor_tensor(out=ym, in0=y0, in1=yf, op=Alu.is_gt)
                    nc.vector.tensor_tensor(out=y0, in0=y0, in1=ym, op=Alu.subtract)
                    nc.vector.tensor_scalar(out=y0, in0=y0, scalar1=62.0, scalar2=None, op0=Alu.min)
                    fx = scr[:, 6:7]
                    fy = scr[:, 7:8]
                    nc.vector.tensor_tensor(out=fx, in0=xf, in1=x0, op=Alu.subtract)
                    nc.vector.tensor_tensor(out=fy, in0=yf, in1=y0, op=Alu.subtract)
                    fx1 = scr[:, 8:9]
                    fy1 = scr[:, 9:10]
                    nc.vector.tensor_scalar(out=fx1, in0=fx, scalar1=-1.0, scalar2=1.0, op0=Alu.mult, op1=Alu.add)
                    nc.vector.tensor_scalar(out=fy1, in0=fy, scalar1=-1.0, scalar2=1.0, op0=Alu.mult, op1=Alu.add)
                    nc.vector.tensor_tensor(out=wkv[:, c * 4 + 0 : c * 4 + 1], in0=fy1, in1=fx1, op=Alu.mult)
                    nc.vector.tensor_tensor(out=wkv[:, c * 4 + 1 : c * 4 + 2], in0=fy1, in1=fx, op=Alu.mult)
                    nc.vector.tensor_tensor(out=wkv[:, c * 4 + 2 : c * 4 + 3], in0=fy, in1=fx1, op=Alu.mult)
                    nc.vector.tensor_tensor(out=wkv[:, c * 4 + 3 : c * 4 + 4], in0=fy, in1=fx, op=Alu.mult)
                    # kv gather indices: y0*64+x0 (+0,+1,+64,+65)
                    ib = scr[:, 10:11]
                    nc.vector.scalar_tensor_tensor(out=ib, in0=y0, scalar=64.0, in1=x0, op0=Alu.mult, op1=Alu.add)
                    for t, offt in enumerate((0.0, 1.0, 64.0, 65.0)):
                        nc.vector.tensor_scalar(
                            out=idxkv[:, c * 4 + t : c * 4 + t + 1], in0=ib,
                            scalar1=offt, scalar2=None, op0=Alu.add,
                        )
                    # bias window coords: cx = 31.5*(1-x), cy = 31.5*(1-y)
                    cxf = scr[:, 0:1]
                    cyf = scr[:, 1:2]
                    nc.vector.tensor_scalar(out=cxf, in0=x, scalar1=-31.5, scalar2=31.5, op0=Alu.mult, op1=Alu.add)
                    nc.vector.tensor_scalar(out=cyf, in0=y, scalar1=-31.5, scalar2=31.5, op0=Alu.mult, op1=Alu.add)
                    fbx = scr[:, 2:3]
                    fby = scr[:, 3:4]
                    x0b = scr[:, 4:5]
                    y0b = scr[:, 5:6]
                    nc.vector.tensor_scalar(out=x0b, in0=cxf, scalar1=8388608.0, scalar2=-8388608.0, op0=Alu.add, op1=Alu.add)
                    nc.vector.tensor_tensor(out=fbx, in0=x0b, in1=cxf, op=Alu.is_gt)
                    nc.vector.tensor_tensor(out=x0b, in0=x0b, in1=fbx, op=Alu.subtract)
                    nc.vector.tensor_scalar(out=y0b, in0=cyf, scalar1=8388608.0, scalar2=-8388608.0, op0=Alu.add, op1=Alu.add)
                    nc.vector.tensor_tensor(out=fby, in0=y0b, in1=cyf, op=Alu.is_gt)
                    nc.vector.tensor_tensor(out=y0b, in0=y0b, in1=fby, op=Alu.subtract)
                    nc.vector.tensor_tensor(out=fbx, in0=cxf, in1=x0b, op=Alu.subtract)
                    nc.vector.tensor_tensor(out=fby, in0=cyf, in1=y0b, op=Alu.subtract)
                    nc.vector.tensor_copy(out=fyb[:, c : c + 1], in_=fby)
                    nc.vector.tensor_scalar(out=dxw[:, c * 2 : c * 2 + 1], in0=fbx, scalar1=-1.0, scalar2=1.0, op0=Alu.mult, op1=Alu.add)
                    nc.vector.tensor_copy(out=dxw[:, c * 2 + 1 : c * 2 + 2], in_=fbx)
                    nc.vector.tensor_tensor(out=dxwf[:, c * 2 : c * 2 + 1], in0=dxw[:, c * 2 : c * 2 + 1], in1=fyb[:, c : c + 1], op=Alu.mult)
                    nc.vector.tensor_tensor(out=dxwf[:, c * 2 + 1 : c * 2 + 2], in0=dxw[:, c * 2 + 1 : c * 2 + 2], in1=fyb[:, c : c + 1], op=Alu.mult)
                    # window index: ((x0b*128)+y0b)*65 + blk_base (+h stride)
                    iw = scr[:, 11:12]
                    nc.vector.scalar_tensor_tensor(out=iw, in0=x0b, scalar=128.0, in1=y0b, op0=Alu.mult, op1=Alu.add)
                    nc.vector.tensor_scalar(
                        out=iw, in0=iw, scalar1=65.0, scalar2=float(blk * TBLK),
                        op0=Alu.mult, op1=Alu.add,
                    )
                    for hh in range(4):
                        nc.vector.tensor_scalar(
                            out=idxw[:, c * 4 + hh : c * 4 + hh + 1], in0=iw,
                            scalar1=float(hh * THEAD), scalar2=None, op0=Alu.add,
                        )

                idxkv_i = sbs2.tile([128, 8], i32, tag="idxkvi")
                nc.vector.tensor_copy(out=idxkv_i[:, :], in_=idxkv[:, :])
                idxw_i = sbs2.tile([128, 8], i32, tag="idxwi")
                nc.vector.tensor_copy(out=idxw_i[:, :], in_=idxw[:, :])

                # diag pairs for DoubleRow bias matmuls, per chunk:
                # dp0[p,0,m]=dxw0[p]*eye, dp0[p,1,m]=dxw0[p]*fyb[p]*eye (x-tap 0)
                # dp1 same with dxw1 (x-tap 1)
                dps = []
                for c in range(2):
                    d0 = sbs2.tile([128, 2, 128], fp8, tag=f"dp0_{c}")
                    d1 = sbs2.tile([128, 2, 128], fp8, tag=f"dp1_{c}")
                    nc.vector.tensor_scalar(out=d0[:, 0, :], in0=eye, scalar1=dxw[:, c * 2 : c * 2 + 1], scalar2=None, op0=Alu.mult)
                    nc.vector.tensor_scalar(out=d0[:, 1, :], in0=eye, scalar1=dxwf[:, c * 2 : c * 2 + 1], scalar2=None, op0=Alu.mult)
                    nc.vector.tensor_scalar(out=d1[:, 0, :], in0=eye, scalar1=dxw[:, c * 2 + 1 : c * 2 + 2], scalar2=None, op0=Alu.mult)
                    nc.vector.tensor_scalar(out=d1[:, 1, :], in0=eye, scalar1=dxwf[:, c * 2 + 1 : c * 2 + 2], scalar2=None, op0=Alu.mult)
                    dps.append((d0, d1))

                return dict(idxw_i=idxw_i, dps=dps, wkv=wkv, idxkv_i=idxkv_i)

            def emit_kv(blk, st):
                wkv, idxkv_i = st["wkv"], st["idxkv_i"]
                # ---------- kv gather + k/v projections ----------
                kvT_ap = BLKS[blk][1]
                pk_wTs1 = wb(blk, 0, 128, rows=65)
                pv_wT1 = wb(blk, 128, 192, rows=65)
                G = sb3.tile([128, 8, 64], f32, tag="G")
                for j in range(8):
                    nc.gpsimd.indirect_dma_start(
                        out=G[:, j, :], out_offset=None, in_=kvT_ap,
                        in_offset=IndirectOffsetOnAxis(ap=idxkv_i[:, j : j + 1], axis=0),
                    )
                xs_b = sb3.tile([65, NS], bf16, tag="xsb")
                nc.vector.memset(xs_b[64:65, :], 1.0)
                for c in range(2):
                    xsT = sb3.tile([128, 64], f32, tag="xsT")
                    nc.vector.tensor_scalar(
                        out=xsT[:, :], in0=G[:, c * 4 + 0, :],
                        scalar1=wkv[:, c * 4 : c * 4 + 1], scalar2=None, op0=Alu.mult,
                    )
                    for t in range(1, 4):
                        nc.vector.scalar_tensor_tensor(
                            out=xsT[:, :], in0=G[:, c * 4 + t, :],
                            scalar=wkv[:, c * 4 + t : c * 4 + t + 1], in1=xsT[:, :],
                            op0=Alu.mult, op1=Alu.add,
                        )
                    xs_p = tps.tile([64, 128], f32, tag="tl")
                    nc.tensor.transpose(out=xs_p[:, :], in_=xsT[:, :], identity=eye)
                    nc.vector.tensor_copy(out=xs_b[0:64, c * 128 : (c + 1) * 128], in_=xs_p[:, :])

                k_p = tps.tile([128, NS], f32, tag="tl")
                nc.tensor.matmul(out=k_p[:, :], lhsT=pk_wTs1, rhs=xs_b[:, :], start=True, stop=True)
                # per-head masked k: [128, c, h, 128] with only rows h*32..+16 nonzero
                k_b4 = sbs2.tile([128, 2, 4, 128], bf16, tag="kb4")
                nc.gpsimd.memset(k_b4[:, :, :, :], 0.0)
                for c in range(2):
                    for h in range(4):
                        nc.vector.tensor_copy(
                            out=k_b4[h * 32 : h * 32 + 16, c, h, :],
                            in_=k_p[h * 32 : h * 32 + 16, c * 128 : (c + 1) * 128],
                        )

                vT1 = sbs2.tile([128, 2, 128], bf16, tag="vT1")
                nc.vector.memset(vT1[:, :, :], 0.0)
                nc.vector.memset(vT1[:, :, :].rearrange("p c (h q) -> p c h q", q=32)[:, :, :, 16:17], 1.0)
                for c in range(2):
                    v_p = tps.tile([128, 64], f32, tag="tl")
                    nc.tensor.matmul(
                        out=v_p[:, :], lhsT=xs_b[:, c * 128 : (c + 1) * 128], rhs=pv_wT1,
                        start=True, stop=True,
                    )
                    vv = vT1[:, c, :].rearrange("p (h q) -> p h q", q=32)
                    nc.vector.tensor_copy(
                        out=vv[:, :, 0:16],
                        in_=v_p[:, :].rearrange("p (h q) -> p h q", q=16),
                    )
                st["k_b4"] = k_b4
                st["vT1"] = vT1

            # ======= attention QK+bias+exp for one head-pair of a block =====
            def emit_qk_pair(blk, st, pg):
                q_b = q_bs[blk]
                idxw_i, dps, k_b4 = st["idxw_i"], st["dps"], st["k_b4"]
                P = ppool.tile([128, 2, 2, HWS], bf16, tag="P")
                st["P"] = P
                for hp in range(2):
                    h = pg * 2 + hp
                    for c in range(2):
                        G2 = gpool.tile([128, 2, 4160], fp8, tag="g2")
                        nc.gpsimd.indirect_dma_start(
                            out=G2[:, 0, :], out_offset=None, in_=tab_d,
                            in_offset=IndirectOffsetOnAxis(ap=idxw_i[:, c * 4 + h : c * 4 + h + 1], axis=0),
                        )
                        nc.gpsimd.indirect_dma_start(
                            out=G2[:, 1, :], out_offset=None, in_=tab_d,
                            in_offset=IndirectOffsetOnAxis(ap=idxw_i[:, c * 4 + h : c * 4 + h + 1], axis=0),
                            element_offset=NTAB,
                        )
                        G2r = G2[:, :, :].rearrange("p t (r q) -> p t r q", q=65)
                        d0, d1 = dps[c]
                        kh = k_b4[:, c, h, :]
                        for wv in range(2):
                            ptds = []
                            for j in range(2):
                                ptd = qkps.tile([128, 1024], f32, tag="qkp")
                                ptds.append(ptd)
                                for half in range(2):
                                    mc = wv * 4 + j * 2 + half
                                    pt = ptd[:, half * 512 : (half + 1) * 512]
                                    nc.tensor.matmul(
                                        out=pt, lhsT=kh,
                                        rhs=q_b[:, mc * 512 : (mc + 1) * 512],
                                        start=True, stop=False,
                                        skip_group_check=True,
                                    )
                                    nc.tensor.matmul(
                                        out=pt, lhsT=d0[:, :, :],
                                        rhs=G2r[:, :, mc * 8 : (mc + 1) * 8, 0:64],
                                        start=False, stop=False, perf_mode=DR,
                                        skip_group_check=True,
                                    )
                                    nc.tensor.matmul(
                                        out=pt, lhsT=d1[:, :, :],
                                        rhs=G2r[:, :, mc * 8 : (mc + 1) * 8, 1:65],
                                        start=False, stop=True, perf_mode=DR,
                                        skip_group_check=True,
                                    )
                            for j in range(2):
                                base = (wv * 4 + j * 2) * 512
                                nc.scalar.activation(
                                    out=P[:, hp, c, base : base + 1024], in_=ptds[j][:, :],
                                    func=Act.Exp, bias=zb[:, :],
                                )

            # =================== AV for one head-pair =======================
            def emit_avpair(blk, st, pg):
                P, vT1 = st["P"], st["vT1"]
                avs = st["avs"]
                for mc in range(8):
                    av = avps.tile([64, 512], f32, tag="avp")
                    for hp in range(2):
                        h = pg * 2 + hp
                        for c in range(2):
                            nc.tensor.matmul(
                                out=av[hp * 32 : (hp + 1) * 32, :],
                                lhsT=vT1[:, c, h * 32 : (h + 1) * 32],
                                rhs=P[:, hp, c, mc * 512 : (mc + 1) * 512],
                                start=(c == 0), stop=(c == 1),
                                skip_group_check=True,
                                tile_position=(0, hp * 32),
                            )
                    dst = avs[pg * 64 : (pg + 1) * 64, mc * 512 : (mc + 1) * 512]
                    nc.vector.tensor_copy(out=dst, in_=av[:, :])

            # ========================== tail ================================
            def emit_tail(blk, st):
                avs = st["avs"]
                R = BLKS[blk][2]
                po_wT_sp = cpb[:, 128 + blk * 64 : 128 + (blk + 1) * 64]
                b4 = cpb[:, 0:128]
                po_b_hi = cp[64:128, 524 + blk : 525 + blk]
                rcp_all = sbt.tile([128, HWS], bf16, tag="rcpa", bufs=1)
                for mc in range(8):
                    sb_p = tps.tile([128, 512], f32, tag="tl")
                    nc.tensor.matmul(out=sb_p[:, :], lhsT=b4, rhs=avs[:, mc * 512 : (mc + 1) * 512], start=True, stop=True)
                    act_raw(rcp_all[:, mc * 512 : (mc + 1) * 512], sb_p[:, :], Act.Reciprocal)
                for mc in range(8):
                    on = sbt.tile([128, 512], bf16, tag="on", bufs=1)
                    nc.vector.tensor_tensor(out=on[:, :], in0=avs[:, mc * 512 : (mc + 1) * 512], in1=rcp_all[:, mc * 512 : (mc + 1) * 512], op=Alu.mult)
                    op = tps.tile([64, 512], f32, tag="tl")
                    nc.tensor.matmul(out=op[:, :], lhsT=po_wT_sp, rhs=on[:, :], start=True, stop=True)
                    nc.vector.scalar_tensor_tensor(
                        out=R[64:128, mc * 512 : (mc + 1) * 512], in0=op[:, :], scalar=po_b_hi,
                        in1=R[64:128, mc * 512 : (mc + 1) * 512], op0=Alu.add, op1=Alu.add,
                    )

            # ========================= main schedule ========================
            emit_qproj(0)
            nc.sync.dma_start(out=cdg[:, :], in_=cdg_d)
            nc.sync.dma_start(out=cdgl[:, :], in_=cdgl_d)
            emit_qproj(1)
            emit_conv(0)
            nc.sync.dma_start(out=wpb[:, :], in_=wpb_d)
            nc.sync.dma_start(out=cpb[:, :], in_=cpb_d)
            nc.sync.dma_start(out=xi1[:, :], in_=xi1_d)
            nc.sync.dma_start(out=xi2[:, :], in_=xi2_d)
            emit_qproj(2)
            emit_conv(1)
            emit_conv(2)

            def emit_block_attn(blk, st):
                st["avs"] = apool.tile([128, HWS], bf16, tag="avs", name="avs")
                emit_qk_pair(blk, st, 0)
                emit_avpair(blk, st, 0)
                emit_qk_pair(blk, st, 1)
                emit_avpair(blk, st, 1)

            nc.sync.dma_start(out=o1_d[0:64, :], in_=xi1[0:64, :])
            nc.sync.dma_start(out=o2_d[0:64, :], in_=xi2[0:64, :])
            st0 = emit_prologue(0)
            st1 = emit_prologue(1)
            emit_kv(0, st0)
            emit_block_attn(0, st0)
            st2 = emit_prologue(2)
            emit_kv(1, st1)
            emit_tail(0, st0)
            nc.sync.dma_start(out=o1_d[64:128, :], in_=xi1[64:128, :])
            emit_block_attn(1, st1)
            emit_kv(2, st2)
            emit_tail(1, st1)
            emit_block_attn(2, st2)
            emit_tail(2, st2)

            nc.sync.dma_start(out=o2_d[64:128, :], in_=xi2[64:128, :])

    nc.compile()
    return nc


def _host_prep(inputs):
    """Build per-core in_maps. inputs: dict of full numpy arrays."""
    import ml_dtypes

    x0, x1, x2 = inputs["x0"], inputs["x1"], inputs["x2"]

    def spread_cols(m):
        # m: [64(in), 64(out)] -> [64(in), 128] with out col h*16+j at h*32+j
        out = np.zeros((m.shape[0], 128), m.dtype)
        for h in range(4):
            out[:, h * 32 : h * 32 + 16] = m[:, h * 16 : (h + 1) * 16]
        return out

    def spread_rows(v):
        # v: [64, k] -> [128, k] with row h*16+j at h*32+j
        out = np.zeros((128,) + v.shape[1:], v.dtype)
        for h in range(4):
            out[h * 32 : h * 32 + 16] = v[h * 16 : (h + 1) * 16]
        return out

    # weight pack f32 (used as f32r): [64, 3*128]  (spread pq_wT)
    wpf = np.zeros((64, 3 * 128), np.float32)
    for b in range(3):
        wpf[:, b * 128 : (b + 1) * 128] = spread_cols(inputs["pq_w"][b].T)
    wpb = np.zeros((65, 3 * 192), ml_dtypes.bfloat16)
    for b in range(3):
        o = b * 192
        pk = np.zeros((65, 128), np.float32)
        pk[0:64] = spread_cols(inputs["pk_w"][b].T * 0.25)
        for h in range(4):
            pk[64, h * 32 : h * 32 + 16] = inputs["pk_b"][b][h * 16 : (h + 1) * 16] * 0.25
        wpb[:, o : o + 128] = pk.astype(ml_dtypes.bfloat16)
        wpb[:64, o + 128 : o + 192] = inputs["pv_w"][b].T.astype(ml_dtypes.bfloat16)
        wpb[64, o + 128 : o + 192] = inputs["pv_b"][b].astype(ml_dtypes.bfloat16)
    # const pack [128, 590]
    cp = np.zeros((128, 590), np.float32)
    cp[:, 0:128] = np.eye(128, dtype=np.float32)
    ys = (np.linspace(0.5, HK - 0.5, HK) / (HK - 1.0)) * 2.0 - 1.0
    cp[0, 128:384] = np.repeat(ys, WK)         # y per n (i-major)
    cp[1, 128:384] = np.tile(ys, HK)           # x per n
    cp[0, 384:512] = 1.0                       # ones1_128
    for h in range(4):
        cp[h * 32 : h * 32 + 16, 520] = 1.0 / 64.0
    for b in range(3):
        cp[:, 521 + b] = spread_rows(inputs["pq_b"][b][:, None])[:, 0]
        cp[64:128, 524 + b] = inputs["po_b"][b]
        bc0 = 527 + b * 21
        cp[:, bc0 : bc0 + 16] = spread_rows(inputs["dw_w"][b].reshape(64, 16))
        cp[:, bc0 + 16] = spread_rows(inputs["dw_b"][b][:, None])[:, 0]
        cp[:, bc0 + 17] = spread_rows(inputs["ln_g"][b][:, None])[:, 0]
        cp[:, bc0 + 18] = spread_rows(inputs["ln_b"][b][:, None])[:, 0]
        cp[:, bc0 + 19 : bc0 + 21] = spread_rows(inputs["pw_w"][b].T)
    cpb = np.zeros((128, 320), ml_dtypes.bfloat16)
    b4 = np.zeros((128, 128), np.float32)
    for h in range(4):
        b4[h * 32 + 16, h * 32 : (h + 1) * 32] = 1.0
    cpb[:, 0:128] = b4.astype(ml_dtypes.bfloat16)
    for b in range(3):
        poT = inputs["po_w"][b].T  # [c, o]
        for h in range(4):
            cpb[h * 32 : h * 32 + 16, 128 + b * 64 : 128 + (b + 1) * 64] = poT[
                h * 16 : (h + 1) * 16
            ].astype(ml_dtypes.bfloat16)
    # depthwise conv diag consts bf16 (hi) + bf16 residual (lo)
    cdg = np.zeros((128, 3 * 16 * 128), ml_dtypes.bfloat16)
    cdgl = np.zeros((128, 3 * 16 * 128), ml_dtypes.bfloat16)
    for b in range(3):
        wsp = spread_rows(inputs["dw_w"][b].reshape(64, 16))  # [128, 16]
        whi = wsp.astype(ml_dtypes.bfloat16).astype(np.float32)
        wlo = wsp - whi
        for t in range(16):
            d = np.zeros((128, 128), np.float32)
            np.fill_diagonal(d, whi[:, t])
            cdg[:, (b * 16 + t) * 128 : (b * 16 + t + 1) * 128] = d.astype(ml_dtypes.bfloat16)
            np.fill_diagonal(d, wlo[:, t])
            cdgl[:, (b * 16 + t) * 128 : (b * 16 + t + 1) * 128] = d.astype(ml_dtypes.bfloat16)
    # rpe slice tables fp8: T windows then D (row-diff) windows
    tab = np.zeros((2, NBLK, NH, 64, TROW, TCOL), ml_dtypes.float8_e4m3)
    rpe = inputs["rpe"]
    for b in range(3):
        for h in range(4):
            pad = np.zeros((129, 128), np.float32)
            pad[0:127, 0:127] = rpe[b, h]
            dif = pad[1:129] - pad[0:128]
            for x0s in range(64):
                tab[0, b, h, x0s] = pad[0:128, x0s : x0s + 65].astype(ml_dtypes.float8_e4m3)
                tab[1, b, h, x0s] = dif[:, x0s : x0s + 65].astype(ml_dtypes.float8_e4m3)
    tab = tab.reshape(-1, 1)

    in_maps = []
    for bb in range(B):
        m = {
            "xi1": np.ascontiguousarray(x1[bb].reshape(C, HWS)),
            "xi2": np.ascontiguousarray(x2[bb].reshape(C, HWS)),
            "xq1": np.ascontiguousarray(x1[bb, :64].reshape(64, HWS)),
            "xq2": np.ascontiguousarray(x2[bb, :64].reshape(64, HWS)),
            "kvT0": np.ascontiguousarray(x0[bb, :64].reshape(64, HWS).T),
            "kvT1": np.ascontiguousarray(x1[bb, :64].reshape(64, HWS).T),
            "wpf": wpf,
            "wpb": wpb,
            "cp": cp,
            "cpb": cpb,
            "cdg": cdg,
            "cdgl": cdgl,
            "rpetab": tab,
        }
        in_maps.append(m)
    return in_maps


def kernel(**inputs):
    from concourse.bass_utils import run_bass_kernel_spmd

    if "nc" not in _CACHE:
        _CACHE["nc"] = _build_graph()
    nc = _CACHE["nc"]
    in_maps = _host_prep(inputs)
    res = run_bass_kernel_spmd(nc, in_maps, core_ids=list(range(8)))
    out = np.zeros((NBLK, B, C, H, W), np.float32)
    out[0] = inputs["x0"]
    for bb in range(B):
        out[1, bb] = res.results[bb]["o1"].reshape(C, H, W)
        out[2, bb] = res.results[bb]["o2"].reshape(C, H, W)
    return out


# revision 22
# speedup vs baseline: 1.1347x; 1.0816x over previous
"""Trainium2 Bass kernel for nn_AttentionTD (3-block deformable attention TD).

Self-contained: hardcodes all shapes. Data-parallel over batch B=8 across the
8 NeuronCores; each core runs the full 3-block DAT stack for one batch element.

v3: precise offset path (f32r qproj + hi/lo bf16 depthwise conv), fp8
DoubleRow bias matmuls folding the fy-interp into the PE, head-pair packed AV
psum, 1024-wide exp chunks, scalar-engine reciprocal, phase-interleaved
emission across the three independent blocks.
"""

import sys

sys.path.insert(0, "/opt/trn_rl_repo")

import numpy as np

# ---------------- problem constants ----------------
B, C, H, W = 8, 128, 64, 64
NCH = 64          # channels per DAT block
NH, HC = 4, 16    # heads, head channels
KS = 4
HWS = H * W       # 4096
HK = WK = 16
NS = HK * WK      # 256 sample points
EPS = 1e-5
NBLK = 3
# rpe slice table geometry: [blk][h][x0 (64)][row (128)][col (65)]
TROW, TCOL = 128, 65
TSLICE = TROW * TCOL          # 8320
THEAD = 64 * TSLICE           # per (blk,h)
TBLK = NH * THEAD
NTAB = NBLK * TBLK

_CACHE = {}


def _build_graph():
    from concourse import bacc, mybir, tile
    import concourse.bass as bass
    from concourse.bass import IndirectOffsetOnAxis

    f32 = mybir.dt.float32
    f32r = mybir.dt.float32r
    bf16 = mybir.dt.bfloat16
    fp8 = mybir.dt.float8e4
    i32 = mybir.dt.int32
    Alu = mybir.AluOpType
    Act = mybir.ActivationFunctionType
    DR = mybir.MatmulPerfMode.DoubleRow

    nc = bacc.Bacc("TRN2", target_bir_lowering=False, debug=False, num_devices=8)

    # ---- dram io ----
    xi1_d = nc.dram_tensor("xi1", [C, HWS], f32, kind="ExternalInput").ap()
    xi2_d = nc.dram_tensor("xi2", [C, HWS], f32, kind="ExternalInput").ap()
    xq1_d = nc.dram_tensor("xq1", [64, HWS], f32r, kind="ExternalInput").ap()
    xq2_d = nc.dram_tensor("xq2", [64, HWS], f32r, kind="ExternalInput").ap()
    kvT0_d = nc.dram_tensor("kvT0", [HWS, NCH], f32, kind="ExternalInput").ap()
    kvT1_d = nc.dram_tensor("kvT1", [HWS, NCH], f32, kind="ExternalInput").ap()
    wpf_d = nc.dram_tensor("wpf", [64, 3 * 128], f32r, kind="ExternalInput").ap()
    wpb_d = nc.dram_tensor("wpb", [65, 3 * 192], bf16, kind="ExternalInput").ap()
    cp_d = nc.dram_tensor("cp", [128, 590], f32, kind="ExternalInput").ap()
    cpb_d = nc.dram_tensor("cpb", [128, 320], bf16, kind="ExternalInput").ap()
    cdg_d = nc.dram_tensor("cdg", [128, 3 * 16 * 128], bf16, kind="ExternalInput").ap()
    cdgl_d = nc.dram_tensor("cdgl", [128, 3 * 16 * 128], bf16, kind="ExternalInput").ap()
    tab_d = nc.dram_tensor("rpetab", [2 * NTAB, 1], fp8, kind="ExternalInput").ap()
    o1_d = nc.dram_tensor("o1", [C, HWS], f32, kind="ExternalOutput").ap()
    o2_d = nc.dram_tensor("o2", [C, HWS], f32, kind="ExternalOutput").ap()

    with tile.TileContext(nc) as tc:
        import contextlib

        ctx = contextlib.ExitStack()
        with ctx:
            cpool = ctx.enter_context(tc.tile_pool(name="const", bufs=1))
            xpool = ctx.enter_context(tc.tile_pool(name="xdata", bufs=1))
            qpool = ctx.enter_context(tc.tile_pool(name="qtiles", bufs=3))
            lpool = ctx.enter_context(tc.tile_pool(name="qlo", bufs=2))
            spool = ctx.enter_context(tc.tile_pool(name="stage", bufs=2))
            ppool = ctx.enter_context(tc.tile_pool(name="probs", bufs=1))
            apool = ctx.enter_context(tc.tile_pool(name="avs", bufs=2))
            gpool = ctx.enter_context(tc.tile_pool(name="wins", bufs=2))
            sb3 = ctx.enter_context(tc.tile_pool(name="blk", bufs=1))
            sbs2 = ctx.enter_context(tc.tile_pool(name="blkstate", bufs=2))
            sba = ctx.enter_context(tc.tile_pool(name="accs", bufs=3))
            sbt = ctx.enter_context(tc.tile_pool(name="tails", bufs=2))
            qkps = ctx.enter_context(tc.tile_pool(name="qk", bufs=2, space="PSUM"))
            avps = ctx.enter_context(tc.tile_pool(name="av", bufs=2, space="PSUM"))
            tps = ctx.enter_context(tc.tile_pool(name="tailp", bufs=2, space="PSUM"))

            # ---- persistent loads ----
            cp = cpool.tile([128, 590], f32, tag="cp")
            nc.sync.dma_start(out=cp[:, :], in_=cp_d)
            wpf = cpool.tile([64, 3 * 128], f32r, tag="wpf")
            nc.sync.dma_start(out=wpf[:, :], in_=wpf_d)
            wpb = cpool.tile([65, 3 * 192], bf16, tag="wpb")
            cpb = cpool.tile([128, 320], bf16, tag="cpb")
            cdg = cpool.tile([128, 3 * 16 * 128], bf16, tag="cdg")
            cdgl = cpool.tile([128, 3 * 16 * 128], bf16, tag="cdgl")
            xi1 = xpool.tile([C, HWS], f32, tag="xi1")
            xi2 = xpool.tile([C, HWS], f32, tag="xi2")

            zb = cpool.tile([128, 1], f32, tag="zb")
            nc.vector.memset(zb[:, :], 0.0)
            epst = cpool.tile([1, 1], f32, tag="epst")
            nc.vector.memset(epst[:, :], EPS)

            eye = cp[:, 0:128]
            ref_yx = cp[0:2, 128:384]          # row0 = y, row1 = x
            ones1_128 = cp[0:1, 384:512]       # [1,128] ones (bcast lhsT)
            ones128_div = cp[0:128, 520:521]   # 1/64 on data rows, 0 on gaps

            def act_raw(out, in_, func):
                eng = nc.scalar
                ins = [eng.lower_ap(in_)]
                for v in (0.0, 1.0, 0.0):
                    ins.append(mybir.ImmediateValue(dtype=mybir.dt.float32, value=v))
                return eng.add_instruction(
                    mybir.InstActivation(
                        name=nc.get_next_instruction_name(), func=func,
                        ins=ins, outs=[eng.lower_ap(out)],
                    )
                )

            def wf(blk, lo, hi):
                return wpf[:, blk * 128 + lo : blk * 128 + hi]

            def wb(blk, lo, hi, rows=64):
                return wpb[0:rows, blk * 192 + lo : blk * 192 + hi]

            BLKS = [(xq1_d, kvT0_d, xi1), (xq2_d, kvT0_d, xi2), (xq2_d, kvT1_d, xi2)]

            # ============ Phase A2: q projections (f32r via staging) ========
            q_bs = [None, None, None]
            q_ls = [None, None, None]

            def emit_qproj(blk):
                XQ_d = BLKS[blk][0]
                pq_wT_sp = wf(blk, 0, 128)
                pq_b_sp = cp[:, 521 + blk : 522 + blk]
                q_b = qpool.tile([128, HWS], bf16, tag="qb")
                q_l = lpool.tile([128, HWS], bf16, tag="ql")
                q_bs[blk] = q_b
                q_ls[blk] = q_l
                for md in range(4):
                    stg = spool.tile([64, 1024], f32r, tag="stg")
                    nc.sync.dma_start(out=stg[:, :], in_=XQ_d[:, md * 1024 : (md + 1) * 1024])
                    qp = qkps.tile([128, 1024], f32, tag="qkp")
                    nc.tensor.matmul(
                        out=qp[:, 0:512], lhsT=pq_wT_sp, rhs=stg[:, 0:512],
                        start=True, stop=True, skip_group_check=True,
                    )
                    nc.tensor.matmul(
                        out=qp[:, 512:1024], lhsT=pq_wT_sp, rhs=stg[:, 512:1024],
                        start=True, stop=True, skip_group_check=True,
                    )
                    nc.scalar.activation(
                        out=q_b[:, md * 1024 : (md + 1) * 1024], in_=qp[:, :],
                        func=Act.Identity, bias=pq_b_sp,
                    )
                    # q_lo = (psum + bias) - q_b  (bf16 residual)
                    nc.vector.scalar_tensor_tensor(
                        out=q_l[:, md * 1024 : (md + 1) * 1024], in0=qp[:, :],
                        scalar=pq_b_sp, in1=q_b[:, md * 1024 : (md + 1) * 1024],
                        op0=Alu.add, op1=Alu.subtract,
                    )

            # ============ Phase A3: depthwise conv (PE diag, hi/lo) =========
            acc_ss = [None, None, None]

            def emit_conv(blk):
                q_b, q_l = q_bs[blk], q_ls[blk]
                q5 = q_b[:, :].rearrange("p (hh a ww b) -> p hh a ww b", hh=16, a=4, ww=16, b=4)
                q5l = q_l[:, :].rearrange("p (hh a ww b) -> p hh a ww b", hh=16, a=4, ww=16, b=4)
                acc_ps = tps.tile([128, NS], f32, tag="tl")
                first = True
                for grp, (lhs, rhsview) in enumerate(((cdg, q5), (cdgl, q5), (cdg, q5l))):
                    for t in range(16):
                        dy, dx = t // 4, t % 4
                        nc.tensor.matmul(
                            out=acc_ps[:, :],
                            lhsT=lhs[:, (blk * 16 + t) * 128 : (blk * 16 + t + 1) * 128],
                            rhs=rhsview[:, :, dy, :, dx],
                            start=first, stop=(grp == 2 and t == 15),
                        )
                        first = False
                dw_b = cp[:, 527 + blk * 21 + 16 : 527 + blk * 21 + 17]
                acc_s = sba.tile([128, NS], f32, tag="dwacc")
                acc_ss[blk] = acc_s
                nc.vector.tensor_scalar(
                    out=acc_s[:, :], in0=acc_ps[:, :], scalar1=dw_b, scalar2=None, op0=Alu.add
                )

            # ============ per-block prologue (LN, GELU, offsets, idx) =======
            def emit_prologue(blk):
                bc0 = 527 + blk * 21
                ln_g = cp[:, bc0 + 17 : bc0 + 18]
                ln_b = cp[:, bc0 + 18 : bc0 + 19]
                pw_wT = cp[:, bc0 + 19 : bc0 + 21]
                acc = acc_ss[blk]

                # ---------- layernorm over channels ----------
                sq = sb3.tile([128, NS], f32, tag="sq")
                nc.vector.tensor_tensor(out=sq[:, :], in0=acc[:, :], in1=acc[:, :], op=Alu.mult)
                mu_p = tps.tile([1, NS], f32, tag="tl")
                nc.tensor.matmul(out=mu_p[:, :], lhsT=ones128_div, rhs=acc[:, :], start=True, stop=True)
                e2_p = tps.tile([1, NS], f32, tag="tl")
                nc.tensor.matmul(out=e2_p[:, :], lhsT=ones128_div, rhs=sq[:, :], start=True, stop=True)
                stats = sb3.tile([1, 2 * NS], f32, tag="stats")
                nc.vector.tensor_copy(out=stats[:, 0:NS], in_=mu_p[:, :])
                mu2 = sb3.tile([1, NS], f32, tag="mu2")
                nc.vector.tensor_tensor(out=mu2[:, :], in0=stats[:, 0:NS], in1=stats[:, 0:NS], op=Alu.mult)
                var = sb3.tile([1, NS], f32, tag="var")
                nc.vector.tensor_tensor(out=var[:, :], in0=e2_p[:, :], in1=mu2[:, :], op=Alu.subtract)
                sd = sb3.tile([1, NS], f32, tag="sd")
                nc.scalar.activation(out=sd[:, :], in_=var[:, :], func=Act.Sqrt, bias=epst[:, :])
                nc.vector.reciprocal(out=stats[:, NS : 2 * NS], in_=sd[:, :])
                bc_p = tps.tile([128, 2 * NS], f32, tag="tl")
                nc.tensor.matmul(out=bc_p[:, :], lhsT=ones1_128, rhs=stats[:, :], start=True, stop=True)
                t1 = sb3.tile([128, NS], f32, tag="t1")
                nc.vector.tensor_tensor(out=t1[:, :], in0=acc[:, :], in1=bc_p[:, 0:NS], op=Alu.subtract)
                nc.vector.tensor_tensor(out=t1[:, :], in0=t1[:, :], in1=bc_p[:, NS : 2 * NS], op=Alu.mult)
                nc.vector.tensor_scalar(
                    out=t1[:, :], in0=t1[:, :], scalar1=ln_g, scalar2=ln_b,
                    op0=Alu.mult, op1=Alu.add,
                )
                # exact GELU via Abramowitz-Stegun erf (|err| <= 1.5e-7)
                ze = sb3.tile([128, NS], f32, tag="ze")
                nc.scalar.activation(out=ze[:, :], in_=t1[:, :], func=Act.Abs,
                                     bias=zb[:, :], scale=0.7071067811865476)
                tt_ = sb3.tile([128, NS], f32, tag="tt")
                nc.vector.tensor_scalar(out=tt_[:, :], in0=ze[:, :], scalar1=0.3275911,
                                        scalar2=1.0, op0=Alu.mult, op1=Alu.add)
                nc.vector.reciprocal(out=tt_[:, :], in_=tt_[:, :])
                poly = sb3.tile([128, NS], f32, tag="poly")
                A = (1.061405429, -1.453152027, 1.421413741, -0.284496736, 0.254829592)
                nc.vector.tensor_scalar(out=poly[:, :], in0=tt_[:, :], scalar1=A[0],
                                        scalar2=A[1], op0=Alu.mult, op1=Alu.add)
                for a_c in A[2:]:
                    nc.vector.tensor_tensor(out=poly[:, :], in0=poly[:, :], in1=tt_[:, :], op=Alu.mult)
                    nc.vector.tensor_scalar(out=poly[:, :], in0=poly[:, :], scalar1=a_c,
                                            scalar2=None, op0=Alu.add)
                nc.vector.tensor_tensor(out=poly[:, :], in0=poly[:, :], in1=tt_[:, :], op=Alu.mult)
                ez = sb3.tile([128, NS], f32, tag="sq")
                nc.vector.tensor_tensor(out=ez[:, :], in0=ze[:, :], in1=ze[:, :], op=Alu.mult)
                nc.scalar.activation(out=ez[:, :], in_=ez[:, :], func=Act.Exp,
                                     bias=zb[:, :], scale=-1.0)
                nc.vector.tensor_tensor(out=poly[:, :], in0=poly[:, :], in1=ez[:, :], op=Alu.mult)
                # erf_abs = 1 - poly
                nc.vector.tensor_scalar(out=poly[:, :], in0=poly[:, :], scalar1=-1.0,
                                        scalar2=1.0, op0=Alu.mult, op1=Alu.add)
                # phi = 0.5 + sign(x)*0.5*erf_abs ; gelu = x*phi
                nc.vector.tensor_tensor(out=ze[:, :], in0=t1[:, :],
                                        in1=zb[:, :].to_broadcast([128, NS]), op=Alu.is_gt)
                nc.vector.tensor_scalar(out=ze[:, :], in0=ze[:, :], scalar1=1.0,
                                        scalar2=-0.5, op0=Alu.mult, op1=Alu.add)
                nc.vector.tensor_tensor(out=poly[:, :], in0=poly[:, :], in1=ze[:, :], op=Alu.mult)
                nc.vector.tensor_scalar(out=poly[:, :], in0=poly[:, :], scalar1=0.5,
                                        scalar2=None, op0=Alu.add)
                gl = sb3.tile([128, NS], f32, tag="tt")
                nc.vector.tensor_tensor(out=gl[:, :], in0=t1[:, :], in1=poly[:, :], op=Alu.mult)

                # ---------- offsets -> positions ----------
                off_p = tps.tile([2, NS], f32, tag="tl")
                nc.tensor.matmul(out=off_p[:, :], lhsT=pw_wT, rhs=gl[:, :], start=True, stop=True)
                pos = sb3.tile([2, NS], f32, tag="pos")
                nc.vector.tensor_tensor(out=pos[:, :], in0=off_p[:, :], in1=ref_yx, op=Alu.add)
                nc.vector.tensor_scalar(
                    out=pos[:, :], in0=pos[:, :], scalar1=1.0, scalar2=-1.0,
                    op0=Alu.min, op1=Alu.max,
                )

                # transpose pos -> [n,(y,x)] per 128-chunk
                posT = sb3.tile([128, 4], f32, tag="posT")  # cols: c0y c0x c1y c1x
                for c in range(2):
                    tp = tps.tile([128, 2], f32, tag="tl")
                    nc.tensor.transpose(
                        out=tp[:, :], in_=pos[:, c * 128 : (c + 1) * 128], identity=eye[0:2, 0:2]
                    )
                    nc.vector.tensor_copy(out=posT[:, c * 2 : c * 2 + 2], in_=tp[:, :])

                # ---------- per-chunk index & weight math ----------
                idxkv = sb3.tile([128, 8], f32, tag="idxkv")
                idxw = sb3.tile([128, 8], f32, tag="idxw")
                fyb = sb3.tile([128, 2], f32, tag="fyb")
                wkv = sbs2.tile([128, 8], f32, tag="wkv")   # w00 w01 w10 w11 per chunk
                dxw = sb3.tile([128, 4], f32, tag="dxw")   # (1-fxb, fxb) per chunk
                dxwf = sb3.tile([128, 4], f32, tag="dxwf")  # dxw * fyb
                scr = sb3.tile([128, 12], f32, tag="scr")

                for c in range(2):
                    y = posT[:, c * 2 : c * 2 + 1]
                    x = posT[:, c * 2 + 1 : c * 2 + 2]
                    # kv pixel coords
                    xf = scr[:, 0:1]
                    yf = scr[:, 1:2]
                    nc.vector.tensor_scalar(out=xf, in0=x, scalar1=1.0, scalar2=31.5, op0=Alu.add, op1=Alu.mult)
                    nc.vector.tensor_scalar(out=yf, in0=y, scalar1=1.0, scalar2=31.5, op0=Alu.add, op1=Alu.mult)
                    xm = scr[:, 2:3]
                    ym = scr[:, 3:4]
                    x0 = scr[:, 4:5]
                    y0 = scr[:, 5:6]
                    # floor via round-to-nearest (+2^23) then subtract (r > x)
                    nc.vector.tensor_scalar(out=x0, in0=xf, scalar1=8388608.0, scalar2=-8388608.0, op0=Alu.add, op1=Alu.add)
                    nc.vector.tensor_tensor(out=xm, in0=x0, in1=xf, op=Alu.is_gt)
                    nc.vector.tensor_tensor(out=x0, in0=x0, in1=xm, op=Alu.subtract)
                    nc.vector.tensor_scalar(out=x0, in0=x0, scalar1=62.0, scalar2=None, op0=Alu.min)
                    nc.vector.tensor_scalar(out=y0, in0=yf, scalar1=8388608.0, scalar2=-8388608.0, op0=Alu.add, op1=Alu.add)
                    nc.vector.tensor_tensor(out=ym, in0=y0, in1=yf, op=Alu.is_gt)
                    nc.vector.tensor_tensor(out=y0, in0=y0, in1=ym, op=Alu.subtract)
                    nc.vector.tensor_scalar(out=y0, in0=y0, scalar1=62.0, scalar2=None, op0=Alu.min)
                    fx = scr[:, 6:7]
                    fy = scr[:, 7:8]
                    nc.vector.tensor_tensor(out=fx, in0=xf, in1=x0, op=Alu.subtract)
                    nc.vector.tensor_tensor(out=fy, in0=yf, in1=y0, op=Alu.subtract)
                    fx1 = scr[:, 8:9]
                    fy1 = scr[:, 9:10]
                    nc.vector.tensor_scalar(out=fx1, in0=fx, scalar1=-1.0, scalar2=1.0, op0=Alu.mult, op1=Alu.add)
                    nc.vector.tensor_scalar(out=fy1, in0=fy, scalar1=-1.0, scalar2=1.0, op0=Alu.mult, op1=Alu.add)
                    nc.vector.tensor_tensor(out=wkv[:, c * 4 + 0 : c * 4 + 1], in0=fy1, in1=fx1, op=Alu.mult)
                    nc.vector.tensor_tensor(out=wkv[:, c * 4 + 1 : c * 4 + 2], in0=fy1, in1=fx, op=Alu.mult)
                    nc.vector.tensor_tensor(out=wkv[:, c * 4 + 2 : c * 4 + 3], in0=fy, in1=fx1, op=Alu.mult)
                    nc.vector.tensor_tensor(out=wkv[:, c * 4 + 3 : c * 4 + 4], in0=fy, in1=fx, op=Alu.mult)
                    # kv gather indices: y0*64+x0 (+0,+1,+64,+65)
                    ib = scr[:, 10:11]
                    nc.vector.scalar_tensor_tensor(out=ib, in0=y0, scalar=64.0, in1=x0, op0=Alu.mult, op1=Alu.add)
                    for t, offt in enumerate((0.0, 1.0, 64.0, 65.0)):
                        nc.vector.tensor_scalar(
                            out=idxkv[:, c * 4 + t : c * 4 + t + 1], in0=ib,
                            scalar1=offt, scalar2=None, op0=Alu.add,
                        )
                    # bias window coords: cx = 31.5*(1-x), cy = 31.5*(1-y)
                    cxf = scr[:, 0:1]
                    cyf = scr[:, 1:2]
                    nc.vector.tensor_scalar(out=cxf, in0=x, scalar1=-31.5, scalar2=31.5, op0=Alu.mult, op1=Alu.add)
                    nc.vector.tensor_scalar(out=cyf, in0=y, scalar1=-31.5, scalar2=31.5, op0=Alu.mult, op1=Alu.add)
                    fbx = scr[:, 2:3]
                    fby = scr[:, 3:4]
                    x0b = scr[:, 4:5]
                    y0b = scr[:, 5:6]
                    nc.vector.tensor_scalar(out=x0b, in0=cxf, scalar1=8388608.0, scalar2=-8388608.0, op0=Alu.add, op1=Alu.add)
                    nc.vector.tensor_tensor(out=fbx, in0=x0b, in1=cxf, op=Alu.is_gt)
                    nc.vector.tensor_tensor(out=x0b, in0=x0b, in1=fbx, op=Alu.subtract)
                    nc.vector.tensor_scalar(out=y0b, in0=cyf, scalar1=8388608.0, scalar2=-8388608.0, op0=Alu.add, op1=Alu.add)
                    nc.vector.tensor_tensor(out=fby, in0=y0b, in1=cyf, op=Alu.is_gt)
                    nc.vector.tensor_tensor(out=y0b, in0=y0b, in1=fby, op=Alu.subtract)
                    nc.vector.tensor_tensor(out=fbx, in0=cxf, in1=x0b, op=Alu.subtract)
                    nc.vector.tensor_tensor(out=fby, in0=cyf, in1=y0b, op=Alu.subtract)
                    nc.vector.tensor_copy(out=fyb[:, c : c + 1], in_=fby)
                    nc.vector.tensor_scalar(out=dxw[:, c * 2 : c * 2 + 1], in0=fbx, scalar1=-1.0, scalar2=1.0, op0=Alu.mult, op1=Alu.add)
                    nc.vector.tensor_copy(out=dxw[:, c * 2 + 1 : c * 2 + 2], in_=fbx)
                    nc.vector.tensor_tensor(out=dxwf[:, c * 2 : c * 2 + 1], in0=dxw[:, c * 2 : c * 2 + 1], in1=fyb[:, c : c + 1], op=Alu.mult)
                    nc.vector.tensor_tensor(out=dxwf[:, c * 2 + 1 : c * 2 + 2], in0=dxw[:, c * 2 + 1 : c * 2 + 2], in1=fyb[:, c : c + 1], op=Alu.mult)
                    # window index: ((x0b*128)+y0b)*65 + blk_base (+h stride)
                    iw = scr[:, 11:12]
                    nc.vector.scalar_tensor_tensor(out=iw, in0=x0b, scalar=128.0, in1=y0b, op0=Alu.mult, op1=Alu.add)
                    nc.vector.tensor_scalar(
                        out=iw, in0=iw, scalar1=65.0, scalar2=float(blk * TBLK),
                        op0=Alu.mult, op1=Alu.add,
                    )
                    for hh in range(4):
                        nc.vector.tensor_scalar(
                            out=idxw[:, c * 4 + hh : c * 4 + hh + 1], in0=iw,
                            scalar1=float(hh * THEAD), scalar2=None, op0=Alu.add,
                        )

                idxkv_i = sbs2.tile([128, 8], i32, tag="idxkvi")
                nc.vector.tensor_copy(out=idxkv_i[:, :], in_=idxkv[:, :])
                idxw_i = sbs2.tile([128, 8], i32, tag="idxwi")
                nc.vector.tensor_copy(out=idxw_i[:, :], in_=idxw[:, :])

                # diag pairs for DoubleRow bias matmuls, per chunk:
                # dp0[p,0,m]=dxw0[p]*eye, dp0[p,1,m]=dxw0[p]*fyb[p]*eye (x-tap 0)
                # dp1 same with dxw1 (x-tap 1)
                dps = []
                for c in range(2):
                    d0 = sbs2.tile([128, 2, 128], fp8, tag=f"dp0_{c}")
                    d1 = sbs2.tile([128, 2, 128], fp8, tag=f"dp1_{c}")
                    nc.vector.tensor_scalar(out=d0[:, 0, :], in0=eye, scalar1=dxw[:, c * 2 : c * 2 + 1], scalar2=None, op0=Alu.mult)
                    nc.vector.tensor_scalar(out=d0[:, 1, :], in0=eye, scalar1=dxwf[:, c * 2 : c * 2 + 1], scalar2=None, op0=Alu.mult)
                    nc.vector.tensor_scalar(out=d1[:, 0, :], in0=eye, scalar1=dxw[:, c * 2 + 1 : c * 2 + 2], scalar2=None, op0=Alu.mult)
                    nc.vector.tensor_scalar(out=d1[:, 1, :], in0=eye, scalar1=dxwf[:, c * 2 + 1 : c * 2 + 2], scalar2=None, op0=Alu.mult)
                    dps.append((d0, d1))

                return dict(idxw_i=idxw_i, dps=dps, wkv=wkv, idxkv_i=idxkv_i)

            def emit_kv(blk, st):
                wkv, idxkv_i = st["wkv"], st["idxkv_i"]
                # ---------- kv gather + k/v projections ----------
                kvT_ap = BLKS[blk][1]
                pk_wTs1 = wb(blk, 0, 128, rows=65)
                pv_wT1 = wb(blk, 128, 192, rows=65)
                G = sb3.tile([128, 8, 64], f32, tag="G")
                for j in range(8):
                    nc.gpsimd.indirect_dma_start(
                        out=G[:, j, :], out_offset=None, in_=kvT_ap,
                        in_offset=IndirectOffsetOnAxis(ap=idxkv_i[:, j : j + 1], axis=0),
                    )
                xs_b = sb3.tile([65, NS], bf16, tag="xsb")
                nc.vector.memset(xs_b[64:65, :], 1.0)
                for c in range(2):
                    xsT = sb3.tile([128, 64], f32, tag="xsT")
                    nc.vector.tensor_scalar(
                        out=xsT[:, :], in0=G[:, c * 4 + 0, :],
                        scalar1=wkv[:, c * 4 : c * 4 + 1], scalar2=None, op0=Alu.mult,
                    )
                    for t in range(1, 4):
                        nc.vector.scalar_tensor_tensor(
                            out=xsT[:, :], in0=G[:, c * 4 + t, :],
                            scalar=wkv[:, c * 4 + t : c * 4 + t + 1], in1=xsT[:, :],
                            op0=Alu.mult, op1=Alu.add,
                        )
                    xs_p = tps.tile([64, 128], f32, tag="tl")
                    nc.tensor.transpose(out=xs_p[:, :], in_=xsT[:, :], identity=eye)
                    nc.vector.tensor_copy(out=xs_b[0:64, c * 128 : (c + 1) * 128], in_=xs_p[:, :])

                k_p = tps.tile([128, NS], f32, tag="tl")
                nc.tensor.matmul(out=k_p[:, :], lhsT=pk_wTs1, rhs=xs_b[:, :], start=True, stop=True)
                # per-head masked k: [128, c, h, 128] with only rows h*32..+16 nonzero
                k_b4 = sbs2.tile([128, 2, 4, 128], bf16, tag="kb4")
                nc.gpsimd.memset(k_b4[:, :, :, :], 0.0)
                for c in range(2):
                    for h in range(4):
                        nc.vector.tensor_copy(
                            out=k_b4[h * 32 : h * 32 + 16, c, h, :],
                            in_=k_p[h * 32 : h * 32 + 16, c * 128 : (c + 1) * 128],
                        )

                vT1 = sbs2.tile([128, 2, 128], bf16, tag="vT1")
                nc.vector.memset(vT1[:, :, :], 0.0)
                nc.vector.memset(vT1[:, :, :].rearrange("p c (h q) -> p c h q", q=32)[:, :, :, 16:17], 1.0)
                for c in range(2):
                    v_p = tps.tile([128, 64], f32, tag="tl")
                    nc.tensor.matmul(
                        out=v_p[:, :], lhsT=xs_b[:, c * 128 : (c + 1) * 128], rhs=pv_wT1,
                        start=True, stop=True,
                    )
                    vv = vT1[:, c, :].rearrange("p (h q) -> p h q", q=32)
                    nc.vector.tensor_copy(
                        out=vv[:, :, 0:16],
                        in_=v_p[:, :].rearrange("p (h q) -> p h q", q=16),
                    )
                st["k_b4"] = k_b4
                st["vT1"] = vT1

            # ======= attention QK+bias+exp for one head-pair of a block =====
            def emit_qk_pair(blk, st, pg):
                q_b = q_bs[blk]
                idxw_i, dps, k_b4 = st["idxw_i"], st["dps"], st["k_b4"]
                P = ppool.tile([128, 2, 2, HWS], bf16, tag="P")
                st["P"] = P
                for hp in range(2):
                    h = pg * 2 + hp
                    for c in range(2):
                        G2 = gpool.tile([128, 2, 4160], fp8, tag="g2")
                        nc.gpsimd.indirect_dma_start(
                            out=G2[:, 0, :], out_offset=None, in_=tab_d,
                            in_offset=IndirectOffsetOnAxis(ap=idxw_i[:, c * 4 + h : c * 4 + h + 1], axis=0),
                        )
                        nc.gpsimd.indirect_dma_start(
                            out=G2[:, 1, :], out_offset=None, in_=tab_d,
                            in_offset=IndirectOffsetOnAxis(ap=idxw_i[:, c * 4 + h : c * 4 + h + 1], axis=0),
                            element_offset=NTAB,
                        )
                        G2r = G2[:, :, :].rearrange("p t (r q) -> p t r q", q=65)
                        d0, d1 = dps[c]
                        kh = k_b4[:, c, h, :]
                        for wv in range(2):
                            ptds = []
                            for j in range(2):
                                ptd = qkps.tile([128, 1024], f32, tag="qkp")
                                ptds.append(ptd)
                                for half in range(2):
                                    mc = wv * 4 + j * 2 + half
                                    pt = ptd[:, half * 512 : (half + 1) * 512]
                                    nc.tensor.matmul(
                                        out=pt, lhsT=kh,
                                        rhs=q_b[:, mc * 512 : (mc + 1) * 512],
                                        start=True, stop=False,
                                        skip_group_check=True,
                                    )
                                    nc.tensor.matmul(
                                        out=pt, lhsT=d0[:, :, :],
                                        rhs=G2r[:, :, mc * 8 : (mc + 1) * 8, 0:64],
                                        start=False, stop=False, perf_mode=DR,
                                        skip_group_check=True,
                                    )
                                    nc.tensor.matmul(
                                        out=pt, lhsT=d1[:, :, :],
                                        rhs=G2r[:, :, mc * 8 : (mc + 1) * 8, 1:65],
                                        start=False, stop=True, perf_mode=DR,
                                        skip_group_check=True,
                                    )
                            for j in range(2):
                                base = (wv * 4 + j * 2) * 512
                                nc.scalar.activation(
                                    out=P[:, hp, c, base : base + 1024], in_=ptds[j][:, :],
                                    func=Act.Exp, bias=zb[:, :],
                                )

            # =================== AV for one head-pair =======================
            def emit_avpair(blk, st, pg):
                P, vT1 = st["P"], st["vT1"]
                avs = st["avs"]
                for mc in range(8):
                    av = avps.tile([64, 512], f32, tag="avp")
                    for hp in range(2):
                        h = pg * 2 + hp
                        for c in range(2):
                            nc.tensor.matmul(
                                out=av[hp * 32 : (hp + 1) * 32, :],
                                lhsT=vT1[:, c, h * 32 : (h + 1) * 32],
                                rhs=P[:, hp, c, mc * 512 : (mc + 1) * 512],
                                start=(c == 0), stop=(c == 1),
                                skip_group_check=True,
                                tile_position=(0, hp * 32),
                            )
                    dst = avs[pg * 64 : (pg + 1) * 64, mc * 512 : (mc + 1) * 512]
                    nc.vector.tensor_copy(out=dst, in_=av[:, :])

            # ========================== tail ================================
            def emit_tail(blk, st, out_d=None):
                avs = st["avs"]
                R = BLKS[blk][2]
                po_wT_sp = cpb[:, 128 + blk * 64 : 128 + (blk + 1) * 64]
                b4 = cpb[:, 0:128]
                po_b_hi = cp[64:128, 524 + blk : 525 + blk]
                rcp_all = sbt.tile([128, HWS], bf16, tag="rcpa", bufs=1)
                for mc in range(8):
                    sb_p = tps.tile([128, 512], f32, tag="tl")
                    nc.tensor.matmul(out=sb_p[:, :], lhsT=b4, rhs=avs[:, mc * 512 : (mc + 1) * 512], start=True, stop=True)
                    act_raw(rcp_all[:, mc * 512 : (mc + 1) * 512], sb_p[:, :], Act.Reciprocal)
                for mc in range(8):
                    on = sbt.tile([128, 512], bf16, tag="on", bufs=1)
                    nc.vector.tensor_tensor(out=on[:, :], in0=avs[:, mc * 512 : (mc + 1) * 512], in1=rcp_all[:, mc * 512 : (mc + 1) * 512], op=Alu.mult)
                    op = tps.tile([64, 512], f32, tag="tl")
                    nc.tensor.matmul(out=op[:, :], lhsT=po_wT_sp, rhs=on[:, :], start=True, stop=True)
                    nc.vector.scalar_tensor_tensor(
                        out=R[64:128, mc * 512 : (mc + 1) * 512], in0=op[:, :], scalar=po_b_hi,
                        in1=R[64:128, mc * 512 : (mc + 1) * 512], op0=Alu.add, op1=Alu.add,
                    )
                    if out_d is not None:
                        nc.sync.dma_start(out=out_d[64:128, mc * 512 : (mc + 1) * 512],
                                          in_=R[64:128, mc * 512 : (mc + 1) * 512])

            # ========================= main schedule ========================
            emit_qproj(0)
            nc.sync.dma_start(out=cdg[:, :], in_=cdg_d)
            nc.sync.dma_start(out=cdgl[:, :], in_=cdgl_d)
            emit_qproj(1)
            emit_conv(0)
            nc.sync.dma_start(out=wpb[:, :], in_=wpb_d)
            nc.sync.dma_start(out=cpb[:, :], in_=cpb_d)
            nc.sync.dma_start(out=xi1[:, :], in_=xi1_d)
            nc.sync.dma_start(out=xi2[:, :], in_=xi2_d)
            emit_qproj(2)
            emit_conv(1)
            emit_conv(2)

            def emit_block_attn(blk, st):
                st["avs"] = apool.tile([128, HWS], bf16, tag="avs", name="avs")
                emit_qk_pair(blk, st, 0)
                emit_avpair(blk, st, 0)
                emit_qk_pair(blk, st, 1)
                emit_avpair(blk, st, 1)

            nc.sync.dma_start(out=o1_d[0:64, :], in_=xi1[0:64, :])
            nc.sync.dma_start(out=o2_d[0:64, :], in_=xi2[0:64, :])
            st0 = emit_prologue(0)
            st1 = emit_prologue(1)
            emit_kv(0, st0)
            emit_block_attn(0, st0)
            st2 = emit_prologue(2)
            emit_kv(1, st1)
            emit_tail(0, st0)
            nc.sync.dma_start(out=o1_d[64:128, :], in_=xi1[64:128, :])
            emit_block_attn(1, st1)
            emit_kv(2, st2)
            emit_tail(1, st1)
            emit_block_attn(2, st2)
            emit_tail(2, st2, out_d=o2_d)

    nc.compile()
    return nc


def _host_prep(inputs):
    """Build per-core in_maps. inputs: dict of full numpy arrays."""
    import ml_dtypes

    x0, x1, x2 = inputs["x0"], inputs["x1"], inputs["x2"]

    def spread_cols(m):
        # m: [64(in), 64(out)] -> [64(in), 128] with out col h*16+j at h*32+j
        out = np.zeros((m.shape[0], 128), m.dtype)
        for h in range(4):
            out[:, h * 32 : h * 32 + 16] = m[:, h * 16 : (h + 1) * 16]
        return out

    def spread_rows(v):
        # v: [64, k] -> [128, k] with row h*16+j at h*32+j
        out = np.zeros((128,) + v.shape[1:], v.dtype)
        for h in range(4):
            out[h * 32 : h * 32 + 16] = v[h * 16 : (h + 1) * 16]
        return out

    # weight pack f32 (used as f32r): [64, 3*128]  (spread pq_wT)
    wpf = np.zeros((64, 3 * 128), np.float32)
    for b in range(3):
        wpf[:, b * 128 : (b + 1) * 128] = spread_cols(inputs["pq_w"][b].T)
    wpb = np.zeros((65, 3 * 192), ml_dtypes.bfloat16)
    for b in range(3):
        o = b * 192
        pk = np.zeros((65, 128), np.float32)
        pk[0:64] = spread_cols(inputs["pk_w"][b].T * 0.25)
        for h in range(4):
            pk[64, h * 32 : h * 32 + 16] = inputs["pk_b"][b][h * 16 : (h + 1) * 16] * 0.25
        wpb[:, o : o + 128] = pk.astype(ml_dtypes.bfloat16)
        wpb[:64, o + 128 : o + 192] = inputs["pv_w"][b].T.astype(ml_dtypes.bfloat16)
        wpb[64, o + 128 : o + 192] = inputs["pv_b"][b].astype(ml_dtypes.bfloat16)
    # const pack [128, 590]
    cp = np.zeros((128, 590), np.float32)
    cp[:, 0:128] = np.eye(128, dtype=np.float32)
    ys = (np.linspace(0.5, HK - 0.5, HK) / (HK - 1.0)) * 2.0 - 1.0
    cp[0, 128:384] = np.repeat(ys, WK)         # y per n (i-major)
    cp[1, 128:384] = np.tile(ys, HK)           # x per n
    cp[0, 384:512] = 1.0                       # ones1_128
    for h in range(4):
        cp[h * 32 : h * 32 + 16, 520] = 1.0 / 64.0
    for b in range(3):
        cp[:, 521 + b] = spread_rows(inputs["pq_b"][b][:, None])[:, 0]
        cp[64:128, 524 + b] = inputs["po_b"][b]
        bc0 = 527 + b * 21
        cp[:, bc0 : bc0 + 16] = spread_rows(inputs["dw_w"][b].reshape(64, 16))
        cp[:, bc0 + 16] = spread_rows(inputs["dw_b"][b][:, None])[:, 0]
        cp[:, bc0 + 17] = spread_rows(inputs["ln_g"][b][:, None])[:, 0]
        cp[:, bc0 + 18] = spread_rows(inputs["ln_b"][b][:, None])[:, 0]
        cp[:, bc0 + 19 : bc0 + 21] = spread_rows(inputs["pw_w"][b].T)
    cpb = np.zeros((128, 320), ml_dtypes.bfloat16)
    b4 = np.zeros((128, 128), np.float32)
    for h in range(4):
        b4[h * 32 + 16, h * 32 : (h + 1) * 32] = 1.0
    cpb[:, 0:128] = b4.astype(ml_dtypes.bfloat16)
    for b in range(3):
        poT = inputs["po_w"][b].T  # [c, o]
        for h in range(4):
            cpb[h * 32 : h * 32 + 16, 128 + b * 64 : 128 + (b + 1) * 64] = poT[
                h * 16 : (h + 1) * 16
            ].astype(ml_dtypes.bfloat16)
    # depthwise conv diag consts bf16 (hi) + bf16 residual (lo)
    cdg = np.zeros((128, 3 * 16 * 128), ml_dtypes.bfloat16)
    cdgl = np.zeros((128, 3 * 16 * 128), ml_dtypes.bfloat16)
    for b in range(3):
        wsp = spread_rows(inputs["dw_w"][b].reshape(64, 16))  # [128, 16]
        whi = wsp.astype(ml_dtypes.bfloat16).astype(np.float32)
        wlo = wsp - whi
        for t in range(16):
            d = np.zeros((128, 128), np.float32)
            np.fill_diagonal(d, whi[:, t])
            cdg[:, (b * 16 + t) * 128 : (b * 16 + t + 1) * 128] = d.astype(ml_dtypes.bfloat16)
            np.fill_diagonal(d, wlo[:, t])
            cdgl[:, (b * 16 + t) * 128 : (b * 16 + t + 1) * 128] = d.astype(ml_dtypes.bfloat16)
    # rpe slice tables fp8: T windows then D (row-diff) windows
    tab = np.zeros((2, NBLK, NH, 64, TROW, TCOL), ml_dtypes.float8_e4m3)
    rpe = inputs["rpe"]
    for b in range(3):
        for h in range(4):
            pad = np.zeros((129, 128), np.float32)
            pad[0:127, 0:127] = rpe[b, h]
            dif = pad[1:129] - pad[0:128]
            for x0s in range(64):
                tab[0, b, h, x0s] = pad[0:128, x0s : x0s + 65].astype(ml_dtypes.float8_e4m3)
                tab[1, b, h, x0s] = dif[:, x0s : x0s + 65].astype(ml_dtypes.float8_e4m3)
    tab = tab.reshape(-1, 1)

    in_maps = []
    for bb in range(B):
        m = {
            "xi1": np.ascontiguousarray(x1[bb].reshape(C, HWS)),
            "xi2": np.ascontiguousarray(x2[bb].reshape(C, HWS)),
            "xq1": np.ascontiguousarray(x1[bb, :64].reshape(64, HWS)),
            "xq2": np.ascontiguousarray(x2[bb, :64].reshape(64, HWS)),
            "kvT0": np.ascontiguousarray(x0[bb, :64].reshape(64, HWS).T),
            "kvT1": np.ascontiguousarray(x1[bb, :64].reshape(64, HWS).T),
            "wpf": wpf,
            "wpb": wpb,
            "cp": cp,
            "cpb": cpb,
            "cdg": cdg,
            "cdgl": cdgl,
            "rpetab": tab,
        }
        in_maps.append(m)
    return in_maps


def kernel(**inputs):
    from concourse.bass_utils import run_bass_kernel_spmd

    if "nc" not in _CACHE:
        _CACHE["nc"] = _build_graph()
    nc = _CACHE["nc"]
    in_maps = _host_prep(inputs)
    res = run_bass_kernel_spmd(nc, in_maps, core_ids=list(range(8)))
    out = np.zeros((NBLK, B, C, H, W), np.float32)
    out[0] = inputs["x0"]
    for bb in range(B):
        out[1, bb] = res.results[bb]["o1"].reshape(C, H, W)
        out[2, bb] = res.results[bb]["o2"].reshape(C, H, W)
    return out


# revision 24
# speedup vs baseline: 1.2175x; 1.0730x over previous
"""Trainium2 Bass kernel for nn_AttentionTD (3-block deformable attention TD).

Self-contained: hardcodes all shapes. Data-parallel over batch B=8 across the
8 NeuronCores; each core runs the full 3-block DAT stack for one batch element.

v3: precise offset path (f32r qproj + hi/lo bf16 depthwise conv), fp8
DoubleRow bias matmuls folding the fy-interp into the PE, head-pair packed AV
psum, 1024-wide exp chunks, scalar-engine reciprocal, phase-interleaved
emission across the three independent blocks.
"""

import sys

sys.path.insert(0, "/opt/trn_rl_repo")

import numpy as np

# ---------------- problem constants ----------------
B, C, H, W = 8, 128, 64, 64
NCH = 64          # channels per DAT block
NH, HC = 4, 16    # heads, head channels
KS = 4
HWS = H * W       # 4096
HK = WK = 16
NS = HK * WK      # 256 sample points
EPS = 1e-5
NBLK = 3
# rpe slice table geometry: [blk][h][x0 (64)][row (128)][col (65)]
TROW, TCOL = 128, 65
TSLICE = TROW * TCOL          # 8320
THEAD = 64 * TSLICE           # per (blk,h)
TBLK = NH * THEAD
NTAB = NBLK * TBLK

_CACHE = {}


def _build_graph():
    from concourse import bacc, mybir, tile
    import concourse.bass as bass
    from concourse.bass import IndirectOffsetOnAxis

    f32 = mybir.dt.float32
    f32r = mybir.dt.float32r
    bf16 = mybir.dt.bfloat16
    fp8 = mybir.dt.float8e4
    i32 = mybir.dt.int32
    Alu = mybir.AluOpType
    Act = mybir.ActivationFunctionType
    DR = mybir.MatmulPerfMode.DoubleRow

    nc = bacc.Bacc("TRN2", target_bir_lowering=False, debug=False, num_devices=8)

    # ---- dram io ----
    xi1_d = nc.dram_tensor("xi1", [C, HWS], f32, kind="ExternalInput").ap()
    xi2_d = nc.dram_tensor("xi2", [C, HWS], f32, kind="ExternalInput").ap()
    xq1_d = nc.dram_tensor("xq1", [64, HWS], f32r, kind="ExternalInput").ap()
    xq2_d = nc.dram_tensor("xq2", [64, HWS], f32r, kind="ExternalInput").ap()
    kvT0_d = nc.dram_tensor("kvT0", [HWS, NCH], f32, kind="ExternalInput").ap()
    kvT1_d = nc.dram_tensor("kvT1", [HWS, NCH], f32, kind="ExternalInput").ap()
    wpf_d = nc.dram_tensor("wpf", [64, 3 * 128], f32r, kind="ExternalInput").ap()
    wpb_d = nc.dram_tensor("wpb", [65, 3 * 192], bf16, kind="ExternalInput").ap()
    cp_d = nc.dram_tensor("cp", [128, 590], f32, kind="ExternalInput").ap()
    cpb_d = nc.dram_tensor("cpb", [128, 320], bf16, kind="ExternalInput").ap()
    cdg_d = nc.dram_tensor("cdg", [128, 3 * 16 * 128], bf16, kind="ExternalInput").ap()
    cdgl_d = nc.dram_tensor("cdgl", [128, 3 * 16 * 128], bf16, kind="ExternalInput").ap()
    tab_d = nc.dram_tensor("rpetab", [2 * NTAB, 1], fp8, kind="ExternalInput").ap()
    o1_d = nc.dram_tensor("o1", [C, HWS], f32, kind="ExternalOutput").ap()
    o2_d = nc.dram_tensor("o2", [C, HWS], f32, kind="ExternalOutput").ap()

    with tile.TileContext(nc) as tc:
        import contextlib

        ctx = contextlib.ExitStack()
        with ctx:
            cpool = ctx.enter_context(tc.tile_pool(name="const", bufs=1))
            xpool = ctx.enter_context(tc.tile_pool(name="xdata", bufs=1))
            qpool = ctx.enter_context(tc.tile_pool(name="qtiles", bufs=3))
            lpool = ctx.enter_context(tc.tile_pool(name="qlo", bufs=2))
            spool = ctx.enter_context(tc.tile_pool(name="stage", bufs=2))
            ppool = ctx.enter_context(tc.tile_pool(name="probs", bufs=1))
            apool = ctx.enter_context(tc.tile_pool(name="avs", bufs=2))
            gpool = ctx.enter_context(tc.tile_pool(name="wins", bufs=2))
            sb3 = ctx.enter_context(tc.tile_pool(name="blk", bufs=1))
            sbs2 = ctx.enter_context(tc.tile_pool(name="blkstate", bufs=2))
            sba = ctx.enter_context(tc.tile_pool(name="accs", bufs=3))
            sbt = ctx.enter_context(tc.tile_pool(name="tails", bufs=2))
            qkps = ctx.enter_context(tc.tile_pool(name="qk", bufs=2, space="PSUM"))
            avps = ctx.enter_context(tc.tile_pool(name="av", bufs=2, space="PSUM"))
            tps = ctx.enter_context(tc.tile_pool(name="tailp", bufs=2, space="PSUM"))

            # ---- persistent loads ----
            cp = cpool.tile([128, 590], f32, tag="cp")
            nc.sync.dma_start(out=cp[:, :], in_=cp_d)
            wpf = cpool.tile([64, 3 * 128], f32r, tag="wpf")
            nc.sync.dma_start(out=wpf[:, :], in_=wpf_d)
            wpb = cpool.tile([65, 3 * 192], bf16, tag="wpb")
            cpb = cpool.tile([128, 320], bf16, tag="cpb")
            cdg = cpool.tile([128, 3 * 16 * 128], bf16, tag="cdg")
            cdgl = cpool.tile([128, 3 * 16 * 128], bf16, tag="cdgl")
            xi1 = xpool.tile([C, HWS], f32, tag="xi1")
            xi2 = xpool.tile([C, HWS], f32, tag="xi2")

            zb = cpool.tile([128, 1], f32, tag="zb")
            nc.vector.memset(zb[:, :], 0.0)
            epst = cpool.tile([1, 1], f32, tag="epst")
            nc.vector.memset(epst[:, :], EPS)

            eye = cp[:, 0:128]
            ref_yx = cp[0:2, 128:384]          # row0 = y, row1 = x
            ones1_128 = cp[0:1, 384:512]       # [1,128] ones (bcast lhsT)
            ones128_div = cp[0:128, 520:521]   # 1/64 on data rows, 0 on gaps

            def act_raw(out, in_, func):
                eng = nc.scalar
                ins = [eng.lower_ap(in_)]
                for v in (0.0, 1.0, 0.0):
                    ins.append(mybir.ImmediateValue(dtype=mybir.dt.float32, value=v))
                return eng.add_instruction(
                    mybir.InstActivation(
                        name=nc.get_next_instruction_name(), func=func,
                        ins=ins, outs=[eng.lower_ap(out)],
                    )
                )

            def wf(blk, lo, hi):
                return wpf[:, blk * 128 + lo : blk * 128 + hi]

            def wb(blk, lo, hi, rows=64):
                return wpb[0:rows, blk * 192 + lo : blk * 192 + hi]

            BLKS = [(xq1_d, kvT0_d, xi1), (xq2_d, kvT0_d, xi2), (xq2_d, kvT1_d, xi2)]

            # ============ Phase A2: q projections (f32r via staging) ========
            q_bs = [None, None, None]
            q_ls = [None, None, None]

            def emit_qproj(blk):
                XQ_d = BLKS[blk][0]
                pq_wT_sp = wf(blk, 0, 128)
                pq_b_sp = cp[:, 521 + blk : 522 + blk]
                q_b = qpool.tile([128, HWS], bf16, tag="qb")
                q_l = lpool.tile([128, HWS], bf16, tag="ql")
                q_bs[blk] = q_b
                q_ls[blk] = q_l
                for md in range(4):
                    stg = spool.tile([64, 1024], f32r, tag="stg")
                    nc.sync.dma_start(out=stg[:, :], in_=XQ_d[:, md * 1024 : (md + 1) * 1024])
                    qp = qkps.tile([128, 1024], f32, tag="qkp")
                    nc.tensor.matmul(
                        out=qp[:, 0:512], lhsT=pq_wT_sp, rhs=stg[:, 0:512],
                        start=True, stop=True, skip_group_check=True,
                    )
                    nc.tensor.matmul(
                        out=qp[:, 512:1024], lhsT=pq_wT_sp, rhs=stg[:, 512:1024],
                        start=True, stop=True, skip_group_check=True,
                    )
                    nc.scalar.activation(
                        out=q_b[:, md * 1024 : (md + 1) * 1024], in_=qp[:, :],
                        func=Act.Identity, bias=pq_b_sp,
                    )
                    # q_lo = (psum + bias) - q_b  (bf16 residual)
                    nc.vector.scalar_tensor_tensor(
                        out=q_l[:, md * 1024 : (md + 1) * 1024], in0=qp[:, :],
                        scalar=pq_b_sp, in1=q_b[:, md * 1024 : (md + 1) * 1024],
                        op0=Alu.add, op1=Alu.subtract,
                    )

            # ============ Phase A3: depthwise conv (PE diag, hi/lo) =========
            acc_ss = [None, None, None]

            def emit_conv(blk):
                q_b, q_l = q_bs[blk], q_ls[blk]
                q5 = q_b[:, :].rearrange("p (hh a ww b) -> p hh a ww b", hh=16, a=4, ww=16, b=4)
                q5l = q_l[:, :].rearrange("p (hh a ww b) -> p hh a ww b", hh=16, a=4, ww=16, b=4)
                acc_ps = tps.tile([128, NS], f32, tag="tl")
                first = True
                for grp, (lhs, rhsview) in enumerate(((cdg, q5), (cdgl, q5), (cdg, q5l))):
                    for t in range(16):
                        dy, dx = t // 4, t % 4
                        nc.tensor.matmul(
                            out=acc_ps[:, :],
                            lhsT=lhs[:, (blk * 16 + t) * 128 : (blk * 16 + t + 1) * 128],
                            rhs=rhsview[:, :, dy, :, dx],
                            start=first, stop=(grp == 2 and t == 15),
                        )
                        first = False
                dw_b = cp[:, 527 + blk * 21 + 16 : 527 + blk * 21 + 17]
                acc_s = sba.tile([128, NS], f32, tag="dwacc")
                acc_ss[blk] = acc_s
                nc.vector.tensor_scalar(
                    out=acc_s[:, :], in0=acc_ps[:, :], scalar1=dw_b, scalar2=None, op0=Alu.add
                )

            # ============ per-block prologue (LN, GELU, offsets, idx) =======
            def emit_prologue(blk):
                bc0 = 527 + blk * 21
                ln_g = cp[:, bc0 + 17 : bc0 + 18]
                ln_b = cp[:, bc0 + 18 : bc0 + 19]
                pw_wT = cp[:, bc0 + 19 : bc0 + 21]
                acc = acc_ss[blk]

                # ---------- layernorm over channels ----------
                sq = sb3.tile([128, NS], f32, tag="sq")
                nc.vector.tensor_tensor(out=sq[:, :], in0=acc[:, :], in1=acc[:, :], op=Alu.mult)
                mu_p = tps.tile([1, NS], f32, tag="tl")
                nc.tensor.matmul(out=mu_p[:, :], lhsT=ones128_div, rhs=acc[:, :], start=True, stop=True)
                e2_p = tps.tile([1, NS], f32, tag="tl")
                nc.tensor.matmul(out=e2_p[:, :], lhsT=ones128_div, rhs=sq[:, :], start=True, stop=True)
                stats = sb3.tile([1, 2 * NS], f32, tag="stats")
                nc.vector.tensor_copy(out=stats[:, 0:NS], in_=mu_p[:, :])
                mu2 = sb3.tile([1, NS], f32, tag="mu2")
                nc.vector.tensor_tensor(out=mu2[:, :], in0=stats[:, 0:NS], in1=stats[:, 0:NS], op=Alu.mult)
                var = sb3.tile([1, NS], f32, tag="var")
                nc.vector.tensor_tensor(out=var[:, :], in0=e2_p[:, :], in1=mu2[:, :], op=Alu.subtract)
                sd = sb3.tile([1, NS], f32, tag="sd")
                nc.scalar.activation(out=sd[:, :], in_=var[:, :], func=Act.Sqrt, bias=epst[:, :])
                nc.vector.reciprocal(out=stats[:, NS : 2 * NS], in_=sd[:, :])
                bc_p = tps.tile([128, 2 * NS], f32, tag="tl")
                nc.tensor.matmul(out=bc_p[:, :], lhsT=ones1_128, rhs=stats[:, :], start=True, stop=True)
                t1 = sb3.tile([128, NS], f32, tag="t1")
                nc.vector.tensor_tensor(out=t1[:, :], in0=acc[:, :], in1=bc_p[:, 0:NS], op=Alu.subtract)
                nc.vector.tensor_tensor(out=t1[:, :], in0=t1[:, :], in1=bc_p[:, NS : 2 * NS], op=Alu.mult)
                nc.vector.tensor_scalar(
                    out=t1[:, :], in0=t1[:, :], scalar1=ln_g, scalar2=ln_b,
                    op0=Alu.mult, op1=Alu.add,
                )
                # exact GELU via Abramowitz-Stegun erf (|err| <= 1.5e-7)
                ze = sb3.tile([128, NS], f32, tag="ze")
                nc.scalar.activation(out=ze[:, :], in_=t1[:, :], func=Act.Abs,
                                     bias=zb[:, :], scale=0.7071067811865476)
                tt_ = sb3.tile([128, NS], f32, tag="tt")
                nc.vector.tensor_scalar(out=tt_[:, :], in0=ze[:, :], scalar1=0.47047,
                                        scalar2=1.0, op0=Alu.mult, op1=Alu.add)
                nc.vector.reciprocal(out=tt_[:, :], in_=tt_[:, :])
                poly = sb3.tile([128, NS], f32, tag="poly")
                A = (0.7478556, -0.0958798, 0.3480242)
                nc.vector.tensor_scalar(out=poly[:, :], in0=tt_[:, :], scalar1=A[0],
                                        scalar2=A[1], op0=Alu.mult, op1=Alu.add)
                nc.vector.tensor_tensor(out=poly[:, :], in0=poly[:, :], in1=tt_[:, :], op=Alu.mult)
                nc.vector.tensor_scalar(out=poly[:, :], in0=poly[:, :], scalar1=A[2],
                                        scalar2=None, op0=Alu.add)
                nc.vector.tensor_tensor(out=poly[:, :], in0=poly[:, :], in1=tt_[:, :], op=Alu.mult)
                ez = sb3.tile([128, NS], f32, tag="sq")
                nc.vector.tensor_tensor(out=ez[:, :], in0=ze[:, :], in1=ze[:, :], op=Alu.mult)
                nc.scalar.activation(out=ez[:, :], in_=ez[:, :], func=Act.Exp,
                                     bias=zb[:, :], scale=-1.0)
                nc.vector.tensor_tensor(out=poly[:, :], in0=poly[:, :], in1=ez[:, :], op=Alu.mult)
                # erf_abs = 1 - poly
                nc.vector.tensor_scalar(out=poly[:, :], in0=poly[:, :], scalar1=-1.0,
                                        scalar2=1.0, op0=Alu.mult, op1=Alu.add)
                # phi = 0.5 + sign(x)*0.5*erf_abs ; gelu = x*phi
                nc.vector.tensor_tensor(out=ze[:, :], in0=t1[:, :],
                                        in1=zb[:, :].to_broadcast([128, NS]), op=Alu.is_gt)
                nc.vector.tensor_scalar(out=ze[:, :], in0=ze[:, :], scalar1=1.0,
                                        scalar2=-0.5, op0=Alu.mult, op1=Alu.add)
                nc.vector.tensor_tensor(out=poly[:, :], in0=poly[:, :], in1=ze[:, :], op=Alu.mult)
                nc.vector.tensor_scalar(out=poly[:, :], in0=poly[:, :], scalar1=0.5,
                                        scalar2=None, op0=Alu.add)
                gl = sb3.tile([128, NS], f32, tag="tt")
                nc.vector.tensor_tensor(out=gl[:, :], in0=t1[:, :], in1=poly[:, :], op=Alu.mult)

                # ---------- offsets -> positions ----------
                off_p = tps.tile([2, NS], f32, tag="tl")
                nc.tensor.matmul(out=off_p[:, :], lhsT=pw_wT, rhs=gl[:, :], start=True, stop=True)
                pos = sb3.tile([2, NS], f32, tag="pos")
                nc.vector.tensor_tensor(out=pos[:, :], in0=off_p[:, :], in1=ref_yx, op=Alu.add)
                nc.vector.tensor_scalar(
                    out=pos[:, :], in0=pos[:, :], scalar1=1.0, scalar2=-1.0,
                    op0=Alu.min, op1=Alu.max,
                )

                # transpose pos -> [n,(y,x)] per 128-chunk
                posT = sb3.tile([128, 4], f32, tag="posT")  # cols: c0y c0x c1y c1x
                for c in range(2):
                    tp = tps.tile([128, 2], f32, tag="tl")
                    nc.tensor.transpose(
                        out=tp[:, :], in_=pos[:, c * 128 : (c + 1) * 128], identity=eye[0:2, 0:2]
                    )
                    nc.vector.tensor_copy(out=posT[:, c * 2 : c * 2 + 2], in_=tp[:, :])

                # ---------- per-chunk index & weight math ----------
                idxkv = sb3.tile([128, 8], f32, tag="idxkv")
                idxw = sb3.tile([128, 8], f32, tag="idxw")
                fyb = sb3.tile([128, 2], f32, tag="fyb")
                wkv = sbs2.tile([128, 8], f32, tag="wkv")   # w00 w01 w10 w11 per chunk
                dxw = sb3.tile([128, 4], f32, tag="dxw")   # (1-fxb, fxb) per chunk
                dxwf = sb3.tile([128, 4], f32, tag="dxwf")  # dxw * fyb
                scr = sb3.tile([128, 12], f32, tag="scr")

                pT = posT[:, :].rearrange("p (c two) -> p c two", two=2)
                yy2 = pT[:, :, 0]   # [128,2] y per chunk
                xx2 = pT[:, :, 1]   # [128,2] x per chunk
                ikv4 = idxkv[:, :].rearrange("p (c f) -> p c f", f=4)
                iw4 = idxw[:, :].rearrange("p (c f) -> p c f", f=4)
                wk4 = wkv[:, :].rearrange("p (c f) -> p c f", f=4)
                dx2 = dxw[:, :].rearrange("p (c f) -> p c f", f=2)
                dxf2 = dxwf[:, :].rearrange("p (c f) -> p c f", f=2)
                s2 = scr[:, :].rearrange("p (k two) -> p k two", two=2)
                xf = s2[:, 0, :]
                yf = s2[:, 1, :]
                nc.vector.tensor_scalar(out=xf, in0=xx2, scalar1=1.0, scalar2=31.5, op0=Alu.add, op1=Alu.mult)
                nc.vector.tensor_scalar(out=yf, in0=yy2, scalar1=1.0, scalar2=31.5, op0=Alu.add, op1=Alu.mult)
                xm = s2[:, 2, :]
                x0 = s2[:, 3, :]
                y0 = s2[:, 4, :]
                # floor via round-to-nearest (+2^23) then subtract (r > x)
                nc.vector.tensor_scalar(out=x0, in0=xf, scalar1=8388608.0, scalar2=-8388608.0, op0=Alu.add, op1=Alu.add)
                nc.vector.tensor_tensor(out=xm, in0=x0, in1=xf, op=Alu.is_gt)
                nc.vector.tensor_tensor(out=x0, in0=x0, in1=xm, op=Alu.subtract)
                nc.vector.tensor_scalar(out=x0, in0=x0, scalar1=62.0, scalar2=None, op0=Alu.min)
                nc.vector.tensor_scalar(out=y0, in0=yf, scalar1=8388608.0, scalar2=-8388608.0, op0=Alu.add, op1=Alu.add)
                nc.vector.tensor_tensor(out=xm, in0=y0, in1=yf, op=Alu.is_gt)
                nc.vector.tensor_tensor(out=y0, in0=y0, in1=xm, op=Alu.subtract)
                nc.vector.tensor_scalar(out=y0, in0=y0, scalar1=62.0, scalar2=None, op0=Alu.min)
                fx = s2[:, 5, :]
                fy = s2[:, 0, :]
                nc.vector.tensor_tensor(out=fx, in0=xf, in1=x0, op=Alu.subtract)
                nc.vector.tensor_tensor(out=fy, in0=yf, in1=y0, op=Alu.subtract)
                fx1 = s2[:, 1, :]
                fy1 = s2[:, 2, :]
                nc.vector.tensor_scalar(out=fx1, in0=fx, scalar1=-1.0, scalar2=1.0, op0=Alu.mult, op1=Alu.add)
                nc.vector.tensor_scalar(out=fy1, in0=fy, scalar1=-1.0, scalar2=1.0, op0=Alu.mult, op1=Alu.add)
                nc.vector.tensor_tensor(out=wk4[:, :, 0], in0=fy1, in1=fx1, op=Alu.mult)
                nc.vector.tensor_tensor(out=wk4[:, :, 1], in0=fy1, in1=fx, op=Alu.mult)
                nc.vector.tensor_tensor(out=wk4[:, :, 2], in0=fy, in1=fx1, op=Alu.mult)
                nc.vector.tensor_tensor(out=wk4[:, :, 3], in0=fy, in1=fx, op=Alu.mult)
                # kv gather indices: y0*64+x0 (+0,+64) -- row-pair gathers
                ib = s2[:, 5, :]
                nc.vector.scalar_tensor_tensor(out=ib, in0=y0, scalar=64.0, in1=x0, op0=Alu.mult, op1=Alu.add)
                nc.vector.tensor_copy(out=ikv4[:, :, 0], in_=ib)
                nc.vector.tensor_scalar(out=ikv4[:, :, 2], in0=ib, scalar1=64.0, scalar2=None, op0=Alu.add)
                # bias window coords: cx = 31.5*(1-x), cy = 31.5*(1-y)
                cxf = s2[:, 0, :]
                cyf = s2[:, 1, :]
                nc.vector.tensor_scalar(out=cxf, in0=xx2, scalar1=-31.5, scalar2=31.5, op0=Alu.mult, op1=Alu.add)
                nc.vector.tensor_scalar(out=cyf, in0=yy2, scalar1=-31.5, scalar2=31.5, op0=Alu.mult, op1=Alu.add)
                fbx = s2[:, 2, :]
                x0b = s2[:, 3, :]
                y0b = s2[:, 4, :]
                nc.vector.tensor_scalar(out=x0b, in0=cxf, scalar1=8388608.0, scalar2=-8388608.0, op0=Alu.add, op1=Alu.add)
                nc.vector.tensor_tensor(out=fbx, in0=x0b, in1=cxf, op=Alu.is_gt)
                nc.vector.tensor_tensor(out=x0b, in0=x0b, in1=fbx, op=Alu.subtract)
                nc.vector.tensor_scalar(out=y0b, in0=cyf, scalar1=8388608.0, scalar2=-8388608.0, op0=Alu.add, op1=Alu.add)
                nc.vector.tensor_tensor(out=fbx, in0=y0b, in1=cyf, op=Alu.is_gt)
                nc.vector.tensor_tensor(out=y0b, in0=y0b, in1=fbx, op=Alu.subtract)
                nc.vector.tensor_tensor(out=fbx, in0=cxf, in1=x0b, op=Alu.subtract)
                nc.vector.tensor_tensor(out=fyb[:, :], in0=cyf, in1=y0b, op=Alu.subtract)
                nc.vector.tensor_scalar(out=dx2[:, :, 0], in0=fbx, scalar1=-1.0, scalar2=1.0, op0=Alu.mult, op1=Alu.add)
                nc.vector.tensor_copy(out=dx2[:, :, 1], in_=fbx)
                nc.vector.tensor_tensor(out=dxf2[:, :, 0], in0=dx2[:, :, 0], in1=fyb[:, :], op=Alu.mult)
                nc.vector.tensor_tensor(out=dxf2[:, :, 1], in0=dx2[:, :, 1], in1=fyb[:, :], op=Alu.mult)
                # window index: ((x0b*128)+y0b)*65 + blk_base (+h stride)
                iw = s2[:, 5, :]
                nc.vector.scalar_tensor_tensor(out=iw, in0=x0b, scalar=128.0, in1=y0b, op0=Alu.mult, op1=Alu.add)
                nc.vector.tensor_scalar(
                    out=iw, in0=iw, scalar1=65.0, scalar2=float(blk * TBLK),
                    op0=Alu.mult, op1=Alu.add,
                )
                for hh in range(4):
                    nc.vector.tensor_scalar(
                        out=iw4[:, :, hh], in0=iw,
                        scalar1=float(hh * THEAD), scalar2=None, op0=Alu.add,
                    )

                idxkv_i = sbs2.tile([128, 8], i32, tag="idxkvi")
                nc.vector.tensor_copy(out=idxkv_i[:, :], in_=idxkv[:, :])
                idxw_i = sbs2.tile([128, 8], i32, tag="idxwi")
                nc.vector.tensor_copy(out=idxw_i[:, :], in_=idxw[:, :])

                # diag pairs for DoubleRow bias matmuls, per chunk:
                # dp0[p,0,m]=dxw0[p]*eye, dp0[p,1,m]=dxw0[p]*fyb[p]*eye (x-tap 0)
                # dp1 same with dxw1 (x-tap 1)
                dps = []
                for c in range(2):
                    d0 = sbs2.tile([128, 2, 128], fp8, tag=f"dp0_{c}")
                    d1 = sbs2.tile([128, 2, 128], fp8, tag=f"dp1_{c}")
                    nc.vector.tensor_scalar(out=d0[:, 0, :], in0=eye, scalar1=dxw[:, c * 2 : c * 2 + 1], scalar2=None, op0=Alu.mult)
                    nc.vector.tensor_scalar(out=d0[:, 1, :], in0=eye, scalar1=dxwf[:, c * 2 : c * 2 + 1], scalar2=None, op0=Alu.mult)
                    nc.vector.tensor_scalar(out=d1[:, 0, :], in0=eye, scalar1=dxw[:, c * 2 + 1 : c * 2 + 2], scalar2=None, op0=Alu.mult)
                    nc.vector.tensor_scalar(out=d1[:, 1, :], in0=eye, scalar1=dxwf[:, c * 2 + 1 : c * 2 + 2], scalar2=None, op0=Alu.mult)
                    dps.append((d0, d1))

                return dict(idxw_i=idxw_i, dps=dps, wkv=wkv, idxkv_i=idxkv_i)

            def emit_kv(blk, st):
                wkv, idxkv_i = st["wkv"], st["idxkv_i"]
                # ---------- kv gather + k/v projections ----------
                kvT_ap = BLKS[blk][1]
                pk_wTs1 = wb(blk, 0, 128, rows=65)
                pv_wT1 = wb(blk, 128, 192, rows=65)
                G = sb3.tile([128, 8, 64], f32, tag="G")
                for c in range(2):
                    for jj in (0, 2):
                        for j2 in range(2):
                            nc.gpsimd.indirect_dma_start(
                                out=G[:, c * 4 + jj + j2, :], out_offset=None, in_=kvT_ap,
                                in_offset=IndirectOffsetOnAxis(ap=idxkv_i[:, c * 4 + jj : c * 4 + jj + 1], axis=0),
                                element_offset=j2 * 64,
                            )
                xs_b = sb3.tile([65, NS], bf16, tag="xsb")
                nc.vector.memset(xs_b[64:65, :], 1.0)
                for c in range(2):
                    xsT = sb3.tile([128, 64], f32, tag="xsT")
                    nc.vector.tensor_scalar(
                        out=xsT[:, :], in0=G[:, c * 4 + 0, :],
                        scalar1=wkv[:, c * 4 : c * 4 + 1], scalar2=None, op0=Alu.mult,
                    )
                    for t in range(1, 4):
                        nc.vector.scalar_tensor_tensor(
                            out=xsT[:, :], in0=G[:, c * 4 + t, :],
                            scalar=wkv[:, c * 4 + t : c * 4 + t + 1], in1=xsT[:, :],
                            op0=Alu.mult, op1=Alu.add,
                        )
                    xs_p = tps.tile([64, 128], f32, tag="tl")
                    nc.tensor.transpose(out=xs_p[:, :], in_=xsT[:, :], identity=eye)
                    nc.vector.tensor_copy(out=xs_b[0:64, c * 128 : (c + 1) * 128], in_=xs_p[:, :])

                k_p = tps.tile([128, NS], f32, tag="tl")
                nc.tensor.matmul(out=k_p[:, :], lhsT=pk_wTs1, rhs=xs_b[:, :], start=True, stop=True)
                # per-head masked k: [128, c, h, 128] with only rows h*32..+16 nonzero
                k_b4 = sbs2.tile([128, 2, 4, 128], bf16, tag="kb4")
                nc.gpsimd.memset(k_b4[:, :, :, :], 0.0)
                for c in range(2):
                    for h in range(4):
                        nc.vector.tensor_copy(
                            out=k_b4[h * 32 : h * 32 + 16, c, h, :],
                            in_=k_p[h * 32 : h * 32 + 16, c * 128 : (c + 1) * 128],
                        )

                vT1 = sbs2.tile([128, 2, 128], bf16, tag="vT1")
                nc.vector.memset(vT1[:, :, :], 0.0)
                nc.vector.memset(vT1[:, :, :].rearrange("p c (h q) -> p c h q", q=32)[:, :, :, 16:17], 1.0)
                for c in range(2):
                    v_p = tps.tile([128, 64], f32, tag="tl")
                    nc.tensor.matmul(
                        out=v_p[:, :], lhsT=xs_b[:, c * 128 : (c + 1) * 128], rhs=pv_wT1,
                        start=True, stop=True,
                    )
                    vv = vT1[:, c, :].rearrange("p (h q) -> p h q", q=32)
                    nc.vector.tensor_copy(
                        out=vv[:, :, 0:16],
                        in_=v_p[:, :].rearrange("p (h q) -> p h q", q=16),
                    )
                st["k_b4"] = k_b4
                st["vT1"] = vT1

            # ======= attention QK+bias+exp for one head-pair of a block =====
            def emit_qk_pair(blk, st, pg):
                q_b = q_bs[blk]
                idxw_i, dps, k_b4 = st["idxw_i"], st["dps"], st["k_b4"]
                P = ppool.tile([128, 2, 2, HWS], bf16, tag="P")
                st["P"] = P
                for hp in range(2):
                    h = pg * 2 + hp
                    for c in range(2):
                        G2 = gpool.tile([128, 2, 4160], fp8, tag="g2")
                        nc.gpsimd.indirect_dma_start(
                            out=G2[:, 0, :], out_offset=None, in_=tab_d,
                            in_offset=IndirectOffsetOnAxis(ap=idxw_i[:, c * 4 + h : c * 4 + h + 1], axis=0),
                        )
                        nc.gpsimd.indirect_dma_start(
                            out=G2[:, 1, :], out_offset=None, in_=tab_d,
                            in_offset=IndirectOffsetOnAxis(ap=idxw_i[:, c * 4 + h : c * 4 + h + 1], axis=0),
                            element_offset=NTAB,
                        )
                        G2r = G2[:, :, :].rearrange("p t (r q) -> p t r q", q=65)
                        d0, d1 = dps[c]
                        kh = k_b4[:, c, h, :]
                        for wv in range(2):
                            ptds = []
                            for j in range(2):
                                ptd = qkps.tile([128, 1024], f32, tag="qkp")
                                ptds.append(ptd)
                                for half in range(2):
                                    mc = wv * 4 + j * 2 + half
                                    pt = ptd[:, half * 512 : (half + 1) * 512]
                                    nc.tensor.matmul(
                                        out=pt, lhsT=kh,
                                        rhs=q_b[:, mc * 512 : (mc + 1) * 512],
                                        start=True, stop=False,
                                        skip_group_check=True,
                                    )
                                    nc.tensor.matmul(
                                        out=pt, lhsT=d0[:, :, :],
                                        rhs=G2r[:, :, mc * 8 : (mc + 1) * 8, 0:64],
                                        start=False, stop=False, perf_mode=DR,
                                        skip_group_check=True,
                                    )
                                    nc.tensor.matmul(
                                        out=pt, lhsT=d1[:, :, :],
                                        rhs=G2r[:, :, mc * 8 : (mc + 1) * 8, 1:65],
                                        start=False, stop=True, perf_mode=DR,
                                        skip_group_check=True,
                                    )
                            for j in range(2):
                                base = (wv * 4 + j * 2) * 512
                                nc.scalar.activation(
                                    out=P[:, hp, c, base : base + 1024], in_=ptds[j][:, :],
                                    func=Act.Exp, bias=zb[:, :],
                                )

            # =================== AV for one head-pair =======================
            def emit_avpair(blk, st, pg):
                P, vT1 = st["P"], st["vT1"]
                avs = st["avs"]
                for mc in range(8):
                    av = avps.tile([64, 512], f32, tag="avp")
                    for hp in range(2):
                        h = pg * 2 + hp
                        for c in range(2):
                            nc.tensor.matmul(
                                out=av[hp * 32 : (hp + 1) * 32, :],
                                lhsT=vT1[:, c, h * 32 : (h + 1) * 32],
                                rhs=P[:, hp, c, mc * 512 : (mc + 1) * 512],
                                start=(c == 0), stop=(c == 1),
                                skip_group_check=True,
                                tile_position=(0, hp * 32),
                            )
                    dst = avs[pg * 64 : (pg + 1) * 64, mc * 512 : (mc + 1) * 512]
                    nc.vector.tensor_copy(out=dst, in_=av[:, :])

            # ========================== tail ================================
            def emit_tail(blk, st, out_d=None):
                avs = st["avs"]
                R = BLKS[blk][2]
                po_wT_sp = cpb[:, 128 + blk * 64 : 128 + (blk + 1) * 64]
                b4 = cpb[:, 0:128]
                po_b_hi = cp[64:128, 524 + blk : 525 + blk]
                rcp_all = sbt.tile([128, HWS], bf16, tag="rcpa", bufs=1)
                for mc in range(8):
                    sb_p = tps.tile([128, 512], f32, tag="tl")
                    nc.tensor.matmul(out=sb_p[:, :], lhsT=b4, rhs=avs[:, mc * 512 : (mc + 1) * 512], start=True, stop=True)
                    act_raw(rcp_all[:, mc * 512 : (mc + 1) * 512], sb_p[:, :], Act.Reciprocal)
                for mc in range(8):
                    on = sbt.tile([128, 512], bf16, tag="on", bufs=1)
                    nc.vector.tensor_tensor(out=on[:, :], in0=avs[:, mc * 512 : (mc + 1) * 512], in1=rcp_all[:, mc * 512 : (mc + 1) * 512], op=Alu.mult)
                    op = tps.tile([64, 512], f32, tag="tl")
                    nc.tensor.matmul(out=op[:, :], lhsT=po_wT_sp, rhs=on[:, :], start=True, stop=True)
                    nc.vector.scalar_tensor_tensor(
                        out=R[64:128, mc * 512 : (mc + 1) * 512], in0=op[:, :], scalar=po_b_hi,
                        in1=R[64:128, mc * 512 : (mc + 1) * 512], op0=Alu.add, op1=Alu.add,
                    )
                    if out_d is not None:
                        nc.sync.dma_start(out=out_d[64:128, mc * 512 : (mc + 1) * 512],
                                          in_=R[64:128, mc * 512 : (mc + 1) * 512])

            # ========================= main schedule ========================
            emit_qproj(0)
            nc.sync.dma_start(out=cdg[:, :], in_=cdg_d)
            nc.sync.dma_start(out=cdgl[:, :], in_=cdgl_d)
            emit_qproj(1)
            emit_conv(0)
            nc.sync.dma_start(out=wpb[:, :], in_=wpb_d)
            nc.sync.dma_start(out=cpb[:, :], in_=cpb_d)
            nc.sync.dma_start(out=xi1[:, :], in_=xi1_d)
            nc.sync.dma_start(out=xi2[:, :], in_=xi2_d)
            emit_qproj(2)
            emit_conv(1)
            emit_conv(2)

            def emit_block_attn(blk, st):
                st["avs"] = apool.tile([128, HWS], bf16, tag="avs", name="avs")
                emit_qk_pair(blk, st, 0)
                emit_avpair(blk, st, 0)
                emit_qk_pair(blk, st, 1)
                emit_avpair(blk, st, 1)

            nc.sync.dma_start(out=o1_d[0:64, :], in_=xi1[0:64, :])
            nc.sync.dma_start(out=o2_d[0:64, :], in_=xi2[0:64, :])
            st0 = emit_prologue(0)
            st1 = emit_prologue(1)
            emit_kv(0, st0)
            emit_block_attn(0, st0)
            st2 = emit_prologue(2)
            emit_kv(1, st1)
            emit_tail(0, st0)
            nc.sync.dma_start(out=o1_d[64:128, :], in_=xi1[64:128, :])
            emit_block_attn(1, st1)
            emit_kv(2, st2)
            emit_tail(1, st1)
            emit_block_attn(2, st2)
            emit_tail(2, st2, out_d=o2_d)

    nc.compile()
    return nc


def _host_prep(inputs):
    """Build per-core in_maps. inputs: dict of full numpy arrays."""
    import ml_dtypes

    x0, x1, x2 = inputs["x0"], inputs["x1"], inputs["x2"]

    def spread_cols(m):
        # m: [64(in), 64(out)] -> [64(in), 128] with out col h*16+j at h*32+j
        out = np.zeros((m.shape[0], 128), m.dtype)
        for h in range(4):
            out[:, h * 32 : h * 32 + 16] = m[:, h * 16 : (h + 1) * 16]
        return out

    def spread_rows(v):
        # v: [64, k] -> [128, k] with row h*16+j at h*32+j
        out = np.zeros((128,) + v.shape[1:], v.dtype)
        for h in range(4):
            out[h * 32 : h * 32 + 16] = v[h * 16 : (h + 1) * 16]
        return out

    # weight pack f32 (used as f32r): [64, 3*128]  (spread pq_wT)
    wpf = np.zeros((64, 3 * 128), np.float32)
    for b in range(3):
        wpf[:, b * 128 : (b + 1) * 128] = spread_cols(inputs["pq_w"][b].T)
    wpb = np.zeros((65, 3 * 192), ml_dtypes.bfloat16)
    for b in range(3):
        o = b * 192
        pk = np.zeros((65, 128), np.float32)
        pk[0:64] = spread_cols(inputs["pk_w"][b].T * 0.25)
        for h in range(4):
            pk[64, h * 32 : h * 32 + 16] = inputs["pk_b"][b][h * 16 : (h + 1) * 16] * 0.25
        wpb[:, o : o + 128] = pk.astype(ml_dtypes.bfloat16)
        wpb[:64, o + 128 : o + 192] = inputs["pv_w"][b].T.astype(ml_dtypes.bfloat16)
        wpb[64, o + 128 : o + 192] = inputs["pv_b"][b].astype(ml_dtypes.bfloat16)
    # const pack [128, 590]
    cp = np.zeros((128, 590), np.float32)
    cp[:, 0:128] = np.eye(128, dtype=np.float32)
    ys = (np.linspace(0.5, HK - 0.5, HK) / (HK - 1.0)) * 2.0 - 1.0
    cp[0, 128:384] = np.repeat(ys, WK)         # y per n (i-major)
    cp[1, 128:384] = np.tile(ys, HK)           # x per n
    cp[0, 384:512] = 1.0                       # ones1_128
    for h in range(4):
        cp[h * 32 : h * 32 + 16, 520] = 1.0 / 64.0
    for b in range(3):
        cp[:, 521 + b] = spread_rows(inputs["pq_b"][b][:, None])[:, 0]
        cp[64:128, 524 + b] = inputs["po_b"][b]
        bc0 = 527 + b * 21
        cp[:, bc0 : bc0 + 16] = spread_rows(inputs["dw_w"][b].reshape(64, 16))
        cp[:, bc0 + 16] = spread_rows(inputs["dw_b"][b][:, None])[:, 0]
        cp[:, bc0 + 17] = spread_rows(inputs["ln_g"][b][:, None])[:, 0]
        cp[:, bc0 + 18] = spread_rows(inputs["ln_b"][b][:, None])[:, 0]
        cp[:, bc0 + 19 : bc0 + 21] = spread_rows(inputs["pw_w"][b].T)
    cpb = np.zeros((128, 320), ml_dtypes.bfloat16)
    b4 = np.zeros((128, 128), np.float32)
    for h in range(4):
        b4[h * 32 + 16, h * 32 : (h + 1) * 32] = 1.0
    cpb[:, 0:128] = b4.astype(ml_dtypes.bfloat16)
    for b in range(3):
        poT = inputs["po_w"][b].T  # [c, o]
        for h in range(4):
            cpb[h * 32 : h * 32 + 16, 128 + b * 64 : 128 + (b + 1) * 64] = poT[
                h * 16 : (h + 1) * 16
            ].astype(ml_dtypes.bfloat16)
    # depthwise conv diag consts bf16 (hi) + bf16 residual (lo)
    cdg = np.zeros((128, 3 * 16 * 128), ml_dtypes.bfloat16)
    cdgl = np.zeros((128, 3 * 16 * 128), ml_dtypes.bfloat16)
    for b in range(3):
        wsp = spread_rows(inputs["dw_w"][b].reshape(64, 16))  # [128, 16]
        whi = wsp.astype(ml_dtypes.bfloat16).astype(np.float32)
        wlo = wsp - whi
        for t in range(16):
            d = np.zeros((128, 128), np.float32)
            np.fill_diagonal(d, whi[:, t])
            cdg[:, (b * 16 + t) * 128 : (b * 16 + t + 1) * 128] = d.astype(ml_dtypes.bfloat16)
            np.fill_diagonal(d, wlo[:, t])
            cdgl[:, (b * 16 + t) * 128 : (b * 16 + t + 1) * 128] = d.astype(ml_dtypes.bfloat16)
    # rpe slice tables fp8: T windows then D (row-diff) windows
    tab = np.zeros((2, NBLK, NH, 64, TROW, TCOL), ml_dtypes.float8_e4m3)
    rpe = inputs["rpe"]
    for b in range(3):
        for h in range(4):
            pad = np.zeros((129, 128), np.float32)
            pad[0:127, 0:127] = rpe[b, h]
            dif = pad[1:129] - pad[0:128]
            for x0s in range(64):
                tab[0, b, h, x0s] = pad[0:128, x0s : x0s + 65].astype(ml_dtypes.float8_e4m3)
                tab[1, b, h, x0s] = dif[:, x0s : x0s + 65].astype(ml_dtypes.float8_e4m3)
    tab = tab.reshape(-1, 1)

    in_maps = []
    for bb in range(B):
        m = {
            "xi1": np.ascontiguousarray(x1[bb].reshape(C, HWS)),
            "xi2": np.ascontiguousarray(x2[bb].reshape(C, HWS)),
            "xq1": np.ascontiguousarray(x1[bb, :64].reshape(64, HWS)),
            "xq2": np.ascontiguousarray(x2[bb, :64].reshape(64, HWS)),
            "kvT0": np.ascontiguousarray(x0[bb, :64].reshape(64, HWS).T),
            "kvT1": np.ascontiguousarray(x1[bb, :64].reshape(64, HWS).T),
            "wpf": wpf,
            "wpb": wpb,
            "cp": cp,
            "cpb": cpb,
            "cdg": cdg,
            "cdgl": cdgl,
            "rpetab": tab,
        }
        in_maps.append(m)
    return in_maps


def kernel(**inputs):
    from concourse.bass_utils import run_bass_kernel_spmd

    if "nc" not in _CACHE:
        _CACHE["nc"] = _build_graph()
    nc = _CACHE["nc"]
    in_maps = _host_prep(inputs)
    res = run_bass_kernel_spmd(nc, in_maps, core_ids=list(range(8)))
    out = np.zeros((NBLK, B, C, H, W), np.float32)
    out[0] = inputs["x0"]
    for bb in range(B):
        out[1, bb] = res.results[bb]["o1"].reshape(C, H, W)
        out[2, bb] = res.results[bb]["o2"].reshape(C, H, W)
    return out
